# revision 1
# baseline (speedup 1.0000x reference)
"""GaussianImage (Cholesky) renderer on 8 trn2 NeuronCores.

Strategy: tile-parallel over the pixel grid (sharding_hint alternative 2).
The 256x256 image is cut into 32x32-pixel tiles (64/frame, 128 total for
T=2).  The host bins gaussians to tiles (pure routing: bbox intersect via a
conservative support radius; outside it exp(-sigma) underflows to 0 in
fp32), pads each tile's gaussian list to 128 slots, and hands every core 16
tile-entries with slot-ordered copies of the RAW inputs.  All math runs on
device:

  per gaussian slot : tanh / sigmoid / conic / quadratic-basis coeffs
  per tile          : sigma = lhsT(6,128)^T @ basis(6,1024)   [TensorE fp32]
                      alpha = Exp(-sigma)                     [ScalarE]
                      img   = w(128,3)^T @ alpha(128,1024)    [TensorE fp32]
                      out   = clamp(img, 0, 1)                [VectorE, fused]

Each pixel is owned by exactly one tile -> no cross-core reduction.
"""

import os
import numpy as np

T, N, H, W = 2, 512, 256, 256
TILE = 32
NT = H // TILE          # 8 tiles per axis
N_CORES = 8
SLOTS = 128
PIX = TILE * TILE       # 1024
SIGMA_CUT = 100.0       # exp(-100) ~ 4e-44: below fp32 denormal resolution

_CACHE = {}


def _build_nc(E, mm2_dtype_name="float32"):
    import concourse.bass as bass
    import concourse.mybir as mybir
    from concourse.tile import TileContext
    import bass_rust

    f32 = mybir.dt.float32
    Alu = mybir.AluOpType
    Act = mybir.ActivationFunctionType

    nc = bass.Bass("TRN2")
    params = nc.dram_tensor("params", [SLOTS, E * 12], f32, kind="ExternalInput")
    basis = nc.dram_tensor("basis", [6, PIX], f32, kind="ExternalInput")
    ident = nc.dram_tensor("ident", [SLOTS, SLOTS], f32, kind="ExternalInput")
    out = nc.dram_tensor("out", [3, E * PIX], f32, kind="ExternalOutput")

    with TileContext(nc) as tc:
        with tc.tile_pool(name="const", bufs=1) as cpool, \
             tc.tile_pool(name="work", bufs=3) as wpool, \
             tc.tile_pool(name="ps_sig", bufs=2, space="PSUM") as ps_sig_pool, \
             tc.tile_pool(name="ps_img", bufs=2, space="PSUM") as ps_img_pool:

            p3 = cpool.tile([SLOTS, E, 12], f32, tag="params")
            bt = cpool.tile([6, PIX], f32, tag="basis")
            it = cpool.tile([SLOTS, SLOTS], f32, tag="ident")
            nc.sync.dma_start(out=p3, in_=params[:].rearrange("p (e k) -> p e k", k=12))
            nc.sync.dma_start(out=bt, in_=basis[:])
            nc.sync.dma_start(out=it, in_=ident[:])

            def sc(tag):
                return cpool.tile([SLOTS, EH], f32, tag=tag, name=tag)

            V = nc.vector
            S = nc.scalar
            EH = E // 2 if E % 2 == 0 else E
            NHALF = E // EH

            ct = cpool.tile([SLOTS, E, 6], f32, tag="coef")
            wt = cpool.tile([SLOTS, E, 3], f32, tag="w")
            f32r = mybir.dt.float32r
            wtr = cpool.tile([SLOTS, E, 3], f32r, tag="wr")
            lhsT = cpool.tile([6, E, SLOTS], f32, tag="lhsT")

            # warm the sigmoid/tanh ACT table set while the params DMA is in
            # flight: the table load (~2.7us) otherwise serializes after it
            warm = cpool.tile([SLOTS, 1], f32, tag="warm")
            nc.gpsimd.memset(warm, 0.0)
            S.activation(warm, warm, Act.Sigmoid)

            for h in range(NHALF):
                es = slice(h * EH, (h + 1) * EH)
                def sc(tag, h=h):
                    return cpool.tile([SLOTS, EH], f32, tag=f"{tag}h{h}", name=f"{tag}h{h}")
                p3h = p3[:, es, :]
                cth = ct[:, es, :]
                mx, my = sc("mx"), sc("my")
                S.activation(mx, p3h[:, :, 0], Act.Tanh)
                S.activation(my, p3h[:, :, 1], Act.Tanh)
                ex, ey = sc("ex"), sc("ey")
                V.scalar_tensor_tensor(out=ex, in0=mx, scalar=0.5 * W, in1=p3h[:, :, 9],
                                       op0=Alu.mult, op1=Alu.subtract)
                V.scalar_tensor_tensor(out=ey, in0=my, scalar=0.5 * H, in1=p3h[:, :, 10],
                                       op0=Alu.mult, op1=Alu.subtract)
                a0, a2 = sc("a0"), sc("a2")
                V.tensor_scalar_add(out=a0, in0=p3h[:, :, 2], scalar1=0.5)
                V.tensor_scalar_add(out=a2, in0=p3h[:, :, 4], scalar1=0.5)
                a1 = p3h[:, :, 3]
                t0, t1, t2, t3 = sc("t0"), sc("t1"), sc("t2"), sc("t3")
                V.tensor_mul(out=t0, in0=a0, in1=a0)
                V.tensor_mul(out=t1, in0=a0, in1=a1)
                V.tensor_mul(out=t2, in0=a1, in1=a1)
                V.tensor_mul(out=t3, in0=a2, in1=a2)
                syy = sc("syy")
                V.tensor_add(out=syy, in0=t2, in1=t3)
                u, v, det, rdet = sc("u"), sc("v"), sc("det"), sc("rdet")
                V.tensor_mul(out=u, in0=t0, in1=syy)
                V.tensor_mul(out=v, in0=t1, in1=t1)
                V.tensor_sub(out=det, in0=u, in1=v)
                V.reciprocal(out=rdet, in_=det)
                ca, cbn, cc = sc("ca"), sc("cbn"), sc("cc")
                V.tensor_mul(out=ca, in0=syy, in1=rdet)
                V.tensor_mul(out=cbn, in0=t1, in1=rdet)
                V.tensor_mul(out=cc, in0=t0, in1=rdet)
                V.tensor_scalar_mul(out=cth[:, :, 0], in0=ca, scalar1=0.5)
                V.tensor_scalar_mul(out=cth[:, :, 1], in0=cbn, scalar1=-1.0)
                V.tensor_scalar_mul(out=cth[:, :, 2], in0=cc, scalar1=0.5)
                m1, m2 = sc("m1"), sc("m2")
                V.tensor_mul(out=m1, in0=ca, in1=ex)
                V.tensor_mul(out=m2, in0=cbn, in1=ey)
                V.tensor_sub(out=cth[:, :, 3], in0=m2, in1=m1)
                m3, m4 = sc("m3"), sc("m4")
                V.tensor_mul(out=m3, in0=cc, in1=ey)
                V.tensor_mul(out=m4, in0=cbn, in1=ex)
                V.tensor_sub(out=cth[:, :, 4], in0=m4, in1=m3)
                exx, exy, eyy = sc("exx"), sc("exy"), sc("eyy")
                V.tensor_mul(out=exx, in0=ex, in1=ex)
                V.tensor_mul(out=exy, in0=ex, in1=ey)
                V.tensor_mul(out=eyy, in0=ey, in1=ey)
                p1, p2, p3b, q = sc("p1"), sc("p2"), sc("p3b"), sc("q")
                V.tensor_mul(out=p1, in0=cth[:, :, 0], in1=exx)
                V.tensor_mul(out=p2, in0=cbn, in1=exy)
                V.tensor_mul(out=p3b, in0=cth[:, :, 2], in1=eyy)
                V.tensor_sub(out=q, in0=p1, in1=p2)
                V.tensor_add(out=cth[:, :, 5], in0=q, in1=p3b)
                osg = sc("osg")
                S.activation(osg, p3h[:, :, 5], Act.Sigmoid)
                S.activation(wt[:, es, :], p3h[:, :, 6:9], Act.Sigmoid)
                for k in range(3):
                    V.tensor_mul(out=wtr[:, es, k], in0=wt[:, es, k], in1=osg)
                tp = ps_img_pool.tile([6 * EH, SLOTS], f32, tag="img", name=f"tp{h}")
                nc.tensor.transpose(tp, cth.rearrange("p e k -> p (e k)"), it)
                tps = cpool.tile([6 * EH, SLOTS], f32, tag=f"tpsh{h}", name=f"tpsh{h}")
                V.tensor_copy(out=tps, in_=tp)
                for j in range(EH):
                    nc.sync.dma_start(out=lhsT[:, h * EH + j, :],
                                      in_=tps[6 * j:6 * j + 6, :])

            st = cpool.tile([3, E * PIX], f32, tag="stage")

            # --- hot loop ---
            for e in range(E):
                sig = ps_sig_pool.tile([SLOTS, PIX], f32, tag="sig")
                lh = lhsT[:, e, :]
                nc.tensor.matmul(sig[:, 0:512], lh, bt[:, 0:512], start=True, stop=True)
                nc.tensor.matmul(sig[:, 512:1024], lh, bt[:, 512:1024], start=True, stop=True)
                alpha = wpool.tile([SLOTS, PIX], f32r, tag="alpha")
                S.activation(alpha, sig, Act.Exp, scale=-1.0)
                img = ps_img_pool.tile([3, PIX], f32, tag="img")
                wre = wtr[:, e, :]
                nc.tensor.matmul(img[:, 0:512], wre, alpha[:, 0:512], start=True, stop=True)
                nc.tensor.matmul(img[:, 512:1024], wre, alpha[:, 512:1024], start=True, stop=True)
                V.tensor_scalar(out=st[:, e * PIX:(e + 1) * PIX], in0=img,
                                scalar1=0.0, scalar2=1.0, op0=Alu.max, op1=Alu.min)
                nc.sync.dma_start(out=out[:, e * PIX:(e + 1) * PIX],
                                  in_=st[:, e * PIX:(e + 1) * PIX])

    bass_rust.generate_event_semaphores(nc)
    return nc


def _bin_entries(xyz, cholesky):
    """Host-side routing: which gaussians overlap which 32x32 tile."""
    means = np.tanh(xyz.astype(np.float64))
    cx = 0.5 * W * (means[..., 0] + 1.0)
    cy = 0.5 * H * (means[..., 1] + 1.0)
    chol = cholesky.astype(np.float64) + np.array([0.5, 0.0, 0.5])
    l0, l1, l2 = chol[..., 0], chol[..., 1], chol[..., 2]
    sxx, sxy, syy = l0 * l0, l0 * l1, l1 * l1 + l2 * l2
    tr, det = sxx + syy, sxx * syy - sxy * sxy
    lam = tr / 2 + np.sqrt(np.maximum(tr * tr / 4 - det, 0.0))
    r = np.sqrt(2.0 * SIGMA_CUT * np.maximum(lam, 0.0)) + 1.0

    entries = []  # (frame, ty, tx, index-list)
    for t in range(T):
        x0 = np.clip(((cx[t] - r[t]) // TILE).astype(int), 0, NT - 1)
        x1 = np.clip(((cx[t] + r[t]) // TILE).astype(int), 0, NT - 1)
        y0 = np.clip(((cy[t] - r[t]) // TILE).astype(int), 0, NT - 1)
        y1 = np.clip(((cy[t] + r[t]) // TILE).astype(int), 0, NT - 1)
        buckets = [[[] for _ in range(NT)] for _ in range(NT)]
        for n in range(N):
            for ty in range(y0[n], y1[n] + 1):
                for tx in range(x0[n], x1[n] + 1):
                    buckets[ty][tx].append(n)
        for ty in range(NT):
            for tx in range(NT):
                assert len(buckets[ty][tx]) <= SLOTS, "tile overflow: >128 gaussians"
                entries.append((t, ty, tx, buckets[ty][tx]))
    return entries


def _ensure_ntff_hook():
    """Provide antenv.axon_hooks (missing in this image) so trace=True works."""
    import sys, types, ctypes, contextlib
    if "antenv.axon_hooks" in sys.modules:
        return
    so_path = "/opt/axon/libaxon_pjrt.so"
    if not os.path.exists(so_path):
        return
    lib = ctypes.CDLL(so_path)
    if not hasattr(lib, "axon_start_nrt_profile"):
        return
    lib.axon_start_nrt_profile.argtypes = [ctypes.POINTER(ctypes.c_int64), ctypes.c_size_t]
    lib.axon_start_nrt_profile.restype = ctypes.c_int64
    lib.axon_stop_nrt_profile.argtypes = [ctypes.c_char_p]
    lib.axon_stop_nrt_profile.restype = ctypes.c_int64

    @contextlib.contextmanager
    def _hook(output_dir, device_ids):
        import jax
        jax.devices()
        if device_ids:
            ids = (ctypes.c_int64 * len(device_ids))(*device_ids)
            rc = lib.axon_start_nrt_profile(ids, len(device_ids))
        else:
            rc = lib.axon_start_nrt_profile(None, 0)
        if rc != 0:
            raise RuntimeError(f"axon_start_nrt_profile rc={rc}")
        try:
            yield
        finally:
            n = lib.axon_stop_nrt_profile(str(output_dir).encode())
            print(f"profile: {n} file(s) written to {output_dir}")

    mod = types.ModuleType("antenv.axon_hooks")
    mod.get_axon_ntff_profile_hook = lambda: _hook
    mod.set_axon_ntff_profile_hook = lambda h: None
    sys.modules["antenv.axon_hooks"] = mod


def kernel(xyz, cholesky, opacity, features_dc):
    from concourse import bass_utils

    xyz = np.asarray(xyz, np.float32)
    cholesky = np.asarray(cholesky, np.float32)
    opacity = np.asarray(opacity, np.float32)
    features_dc = np.asarray(features_dc, np.float32)

    entries = _bin_entries(xyz, cholesky)
    E = (len(entries) + N_CORES - 1) // N_CORES

    # per-core packed params: (128, E, 12) -> flat (128, E*12)
    in_maps = []
    gx = np.arange(PIX, dtype=np.float32) % TILE
    gy = np.arange(PIX, dtype=np.float32) // TILE
    basis = np.stack([gx * gx, gx * gy, gy * gy, gx, gy, np.ones(PIX, np.float32)]).astype(np.float32)
    ident = np.eye(SLOTS, dtype=np.float32)
    for c in range(N_CORES):
        pm = np.zeros((SLOTS, E, 12), np.float32)
        pm[:, :, 5] = -100.0  # dummy slots: sigmoid(opacity) ~ 0
        for ei in range(E):
            k = c * E + ei
            if k >= len(entries):
                continue
            t, ty, tx, idxs = entries[k]
            ns = len(idxs)
            if ns:
                idxs = np.asarray(idxs)
                pm[:ns, ei, 0:2] = xyz[t, idxs]
                pm[:ns, ei, 2:5] = cholesky[t, idxs]
                pm[:ns, ei, 5] = opacity[idxs, 0]
                pm[:ns, ei, 6:9] = features_dc[idxs]
            pm[:, ei, 9] = tx * TILE - 0.5 * W
            pm[:, ei, 10] = ty * TILE - 0.5 * H
        in_maps.append({"params": pm.reshape(SLOTS, E * 12),
                        "basis": basis, "ident": ident})

    if E not in _CACHE:
        _CACHE[E] = _build_nc(E)
    nc = _CACHE[E]

    trace = bool(int(os.environ.get("GS_TRACE", "0")))
    if trace:
        _ensure_ntff_hook()
    res = bass_utils.run_bass_kernel_spmd(
        nc, in_maps, core_ids=list(range(N_CORES)), trace=trace)
    kernel.last_result = res

    img = np.zeros((T, 3, H, W), np.float32)
    for c in range(N_CORES):
        o = res.results[c]["out"].reshape(3, E, TILE, TILE)
        for ei in range(E):
            k = c * E + ei
            if k >= len(entries):
                continue
            t, ty, tx, _ = entries[k]
            img[t, :, ty * TILE:(ty + 1) * TILE, tx * TILE:(tx + 1) * TILE] = o[:, ei]
    return img



# revision 8
# speedup vs baseline: 1.9832x; 1.9832x over previous
"""GaussianImage (Cholesky) renderer on 8 trn2 NeuronCores.

Strategy: tile-parallel over the pixel grid.  The 256x256 image is cut
into 32x32-pixel tiles (64/frame, 128 total for T=2).  The host bins
gaussians to tiles (bbox intersect via a conservative support radius;
outside it exp(-sigma) underflows to 0 in fp32), pads each tile's
gaussian list to 128 slots, and precomputes per-slot quadratic
coefficients of sigma in local tile coordinates:

  sigma(gx,gy) = A gx^2 + B gx gy + C gy^2 + D gx + E gy + F

Each coefficient is split hi/lo into two fp16 values (lo pre-scaled by
2^11 to stay in fp16 normal range; the matching basis rows are scaled
by 2^-11), giving ~fp32-accurate sigma from a single K=12 fp16 matmul
that runs at 4x the fp32 rate on the PE array.  Per tile-entry:

  sigma = lhsT(12,128)^T @ basis(12,1024)      [TensorE fp16, fp32 PSUM]
  alpha = Exp(-sigma)                          [ScalarE -> fp16]
  img   = w(128,3)^T @ alpha(128,1024)         [TensorE fp16, fp32 PSUM]

Images for 4 consecutive entries land in one PSUM tile at partition
offsets 0/32/64/96 (PE column-group tiling) and are DMA'd out unclamped
in a single strided transfer; the final clip(0,1) runs on the host
during unsharding.  Each pixel is owned by exactly one tile -> no
cross-core reduction.
"""

import os
import numpy as np

T, N, H, W = 2, 512, 256, 256
TILE = 32
NT = H // TILE          # 8 tiles per axis
N_CORES = 8
SLOTS = 128
PIX = TILE * TILE       # 1024
SIGMA_CUT = 100.0       # exp(-100) ~ 4e-44: below fp32 denormal resolution
LO_SCALE = 2048.0       # 2^11: keeps lo-half fp16 coefficients normal

_CACHE = {}


def _build_nc(E):
    import concourse.bass as bass
    import concourse.mybir as mybir
    from concourse.tile import TileContext
    import bass_rust

    f32 = mybir.dt.float32
    f16 = mybir.dt.float16
    Act = mybir.ActivationFunctionType

    Alu = mybir.AluOpType
    G = E // 4  # img PSUM groups of 4 entries

    nc = bass.Bass("TRN2")
    lhsT = nc.dram_tensor("lhsT", [12, E * SLOTS], f16, kind="ExternalInput")
    wtr = nc.dram_tensor("wtr", [SLOTS, E * 3], f16, kind="ExternalInput")
    basis = nc.dram_tensor("basis", [12, PIX], f16, kind="ExternalInput")
    out = nc.dram_tensor("out", [G, SLOTS, PIX], f16, kind="ExternalOutput")

    with TileContext(nc) as tc:
        with tc.tile_pool(name="const", bufs=1) as cpool, \
             tc.tile_pool(name="alpha", bufs=3) as apool, \
             tc.tile_pool(name="ps_sig", bufs=2, space="PSUM") as pss, \
             tc.tile_pool(name="ps_img", bufs=2, space="PSUM") as psi:

            S = nc.scalar

            # preload the Exp activation table while input DMAs are in
            # flight (the table load ~1.3us otherwise serializes with the
            # first real exp)
            warm = cpool.tile([SLOTS, 1], f32, tag="warm")
            nc.gpsimd.memset(warm, 0.0)
            S.activation(warm, warm, Act.Exp)

            lt = cpool.tile([12, E * SLOTS], f16, tag="lhsT")
            wt = cpool.tile([SLOTS, E * 3], f16, tag="wtr")
            bt = cpool.tile([12, PIX], f16, tag="basis")
            nc.sync.dma_start(out=lt, in_=lhsT[:])
            nc.sync.dma_start(out=wt, in_=wtr[:])
            nc.sync.dma_start(out=bt, in_=basis[:])

            alphas = {}
            imgt = {}

            def emit_img(e):
                g, i = divmod(e, 4)
                if i == 0:
                    imgt[g] = psi.tile([SLOTS, PIX], f32, tag="img",
                                       name=f"img{g}")
                t = imgt[g]
                al = alphas.pop(e)
                wre = wt[:, 3 * e:3 * e + 3]
                nc.tensor.matmul(t[32 * i:32 * i + 3, 0:512], wre,
                                 al[:, 0:512], start=True, stop=True,
                                 tile_position=(0, 32 * i))
                nc.tensor.matmul(t[32 * i:32 * i + 3, 512:1024], wre,
                                 al[:, 512:1024], start=True, stop=True,
                                 tile_position=(0, 32 * i))
                if i == 3:
                    # fused clamp + PSUM->SBUF fp16 copy for the whole
                    # 4-entry group (one DVE op: cost is cols, not rows)
                    st = apool.tile([SLOTS, PIX], f16, tag="st",
                                    name=f"st{g}")
                    nc.vector.tensor_scalar(out=st, in0=t, scalar1=0.0,
                                            scalar2=1.0, op0=Alu.max,
                                            op1=Alu.min)
                    # NOTE: a two-level partition AP ("(a r) f -> a r f"
                    # then slicing r) mis-lowers in DMA codegen (only the
                    # first row of each 32-block transfers), so ship the
                    # whole clamped tile and slice on the host.
                    nc.sync.dma_start(out=out[g], in_=st)

            for e in range(E):
                sig = pss.tile([SLOTS, PIX], f32, tag="sig", name=f"sig{e}")
                lh = lt[:, SLOTS * e:SLOTS * (e + 1)]
                nc.tensor.matmul(sig[:, 0:512], lh, bt[:, 0:512],
                                 start=True, stop=True)
                nc.tensor.matmul(sig[:, 512:1024], lh, bt[:, 512:1024],
                                 start=True, stop=True)
                # software pipeline: img matmuls of the previous entry go
                # between this entry's sig matmuls and its exp, so the PE
                # never waits on the ScalarE exp of the current entry
                if e > 0:
                    emit_img(e - 1)
                al = apool.tile([SLOTS, PIX], f16, tag="alpha", name=f"al{e}")
                S.activation(al, sig, Act.Exp, scale=-1.0)
                alphas[e] = al
            emit_img(E - 1)

    bass_rust.generate_event_semaphores(nc)
    return nc


def _bin_entries(cx, cy, lam):
    """Host-side routing: which gaussians overlap which 32x32 tile."""
    r = np.sqrt(2.0 * SIGMA_CUT * np.maximum(lam, 0.0)) + 1.0

    entries = []  # (frame, ty, tx, index-list)
    for t in range(T):
        x0 = np.clip(((cx[t] - r[t]) // TILE).astype(int), 0, NT - 1)
        x1 = np.clip(((cx[t] + r[t]) // TILE).astype(int), 0, NT - 1)
        y0 = np.clip(((cy[t] - r[t]) // TILE).astype(int), 0, NT - 1)
        y1 = np.clip(((cy[t] + r[t]) // TILE).astype(int), 0, NT - 1)
        buckets = [[[] for _ in range(NT)] for _ in range(NT)]
        for n in range(N):
            for ty in range(y0[n], y1[n] + 1):
                for tx in range(x0[n], x1[n] + 1):
                    buckets[ty][tx].append(n)
        for ty in range(NT):
            for tx in range(NT):
                assert len(buckets[ty][tx]) <= SLOTS, "tile overflow: >128 gaussians"
                entries.append((t, ty, tx, buckets[ty][tx]))
    return entries


def _ensure_ntff_hook():
    """Provide antenv.axon_hooks (missing in this image) so trace=True works."""
    import sys, types, ctypes, contextlib
    if "antenv.axon_hooks" in sys.modules:
        return
    so_path = "/opt/axon/libaxon_pjrt.so"
    if not os.path.exists(so_path):
        return
    lib = ctypes.CDLL(so_path)
    if not hasattr(lib, "axon_start_nrt_profile"):
        return
    lib.axon_start_nrt_profile.argtypes = [ctypes.POINTER(ctypes.c_int64), ctypes.c_size_t]
    lib.axon_start_nrt_profile.restype = ctypes.c_int64
    lib.axon_stop_nrt_profile.argtypes = [ctypes.c_char_p]
    lib.axon_stop_nrt_profile.restype = ctypes.c_int64

    @contextlib.contextmanager
    def _hook(output_dir, device_ids):
        import jax
        jax.devices()
        if device_ids:
            ids = (ctypes.c_int64 * len(device_ids))(*device_ids)
            rc = lib.axon_start_nrt_profile(ids, len(device_ids))
        else:
            rc = lib.axon_start_nrt_profile(None, 0)
        if rc != 0:
            raise RuntimeError(f"axon_start_nrt_profile rc={rc}")
        try:
            yield
        finally:
            n = lib.axon_stop_nrt_profile(str(output_dir).encode())
            print(f"profile: {n} file(s) written to {output_dir}")

    mod = types.ModuleType("antenv.axon_hooks")
    mod.get_axon_ntff_profile_hook = lambda: _hook
    mod.set_axon_ntff_profile_hook = lambda h: None
    sys.modules["antenv.axon_hooks"] = mod


def _split16(c):
    """Split float64 array c into (hi, lo) fp16 with lo pre-scaled by 2^11."""
    hi = c.astype(np.float16)
    lo = ((c - hi.astype(np.float64)) * LO_SCALE).astype(np.float16)
    return hi, lo


def kernel(xyz, cholesky, opacity, features_dc):
    from concourse import bass_utils

    xyz = np.asarray(xyz, np.float32)
    cholesky = np.asarray(cholesky, np.float32)
    opacity = np.asarray(opacity, np.float32)
    features_dc = np.asarray(features_dc, np.float32)

    # ---- host precompute (float64): projection, conic, binning ----
    means = np.tanh(xyz.astype(np.float64))
    cx = 0.5 * W * (means[..., 0] + 1.0)                    # (T,N)
    cy = 0.5 * H * (means[..., 1] + 1.0)
    chol = cholesky.astype(np.float64) + np.array([0.5, 0.0, 0.5])
    l0, l1, l2 = chol[..., 0], chol[..., 1], chol[..., 2]
    sxx, sxy, syy = l0 * l0, l0 * l1, l1 * l1 + l2 * l2
    det = sxx * syy - sxy * sxy
    ca, cb, cc = syy / det, -sxy / det, sxx / det           # conic (T,N)
    tr = sxx + syy
    lam = tr / 2 + np.sqrt(np.maximum(tr * tr / 4 - det, 0.0))

    colors = 1.0 / (1.0 + np.exp(-features_dc.astype(np.float64)))   # (N,3)
    opac = 1.0 / (1.0 + np.exp(-opacity.astype(np.float64)[:, 0]))   # (N,)
    w3 = colors * opac[:, None]                                      # (N,3)

    entries = _bin_entries(cx, cy, lam)
    E = (len(entries) + N_CORES - 1) // N_CORES

    # fp16 quadratic basis over local 32x32 pixels; rows 6-11 are the
    # lo-coefficient rows, scaled by 2^-11 (power of two: still exact)
    gx = np.arange(PIX, dtype=np.float64) % TILE
    gy = np.arange(PIX, dtype=np.float64) // TILE
    b6 = np.stack([gx * gx, gx * gy, gy * gy, gx, gy, np.ones(PIX)])
    basis = np.concatenate([b6, b6 / LO_SCALE]).astype(np.float16)

    in_maps = []
    for c in range(N_CORES):
        lm = np.zeros((12, E * SLOTS), np.float16)
        wm = np.zeros((SLOTS, E * 3), np.float16)
        for ei in range(E):
            k = c * E + ei
            if k >= len(entries):
                continue
            t, ty, tx, idxs = entries[k]
            ns = len(idxs)
            if not ns:
                continue
            idxs = np.asarray(idxs)
            ex = cx[t, idxs] - tx * TILE
            ey = cy[t, idxs] - ty * TILE
            a_, b_, c_ = ca[t, idxs], cb[t, idxs], cc[t, idxs]
            coef = np.stack([
                0.5 * a_,
                b_,
                0.5 * c_,
                -(a_ * ex + b_ * ey),
                -(b_ * ex + c_ * ey),
                0.5 * (a_ * ex * ex + c_ * ey * ey) + b_ * ex * ey,
            ])                                               # (6, ns)
            hi, lo = _split16(coef)
            s = slice(SLOTS * ei, SLOTS * ei + ns)
            lm[0:6, s] = hi
            lm[6:12, s] = lo
            wm[:ns, 3 * ei:3 * ei + 3] = w3[idxs].astype(np.float16)
        in_maps.append({"lhsT": lm, "wtr": wm, "basis": basis})

    if E not in _CACHE:
        _CACHE[E] = _build_nc(E)
    nc = _CACHE[E]

    trace = bool(int(os.environ.get("GS_TRACE", "0")))
    if trace:
        _ensure_ntff_hook()
    res = bass_utils.run_bass_kernel_spmd(
        nc, in_maps, core_ids=list(range(N_CORES)), trace=trace)
    kernel.last_result = res

    img = np.zeros((T, 3, H, W), np.float32)
    for c in range(N_CORES):
        # out[g] is the full 128-partition clamp tile; entry i of group g
        # lives at partitions 32i..32i+2
        o = res.results[c]["out"].reshape(E // 4, 4, 32, PIX)[:, :, 0:3, :]
        o = o.reshape(E, 3, TILE, TILE)
        for ei in range(E):
            k = c * E + ei
            if k >= len(entries):
                continue
            t, ty, tx, _ = entries[k]
            img[t, :, ty * TILE:(ty + 1) * TILE, tx * TILE:(tx + 1) * TILE] = o[ei]
    return np.clip(img, 0.0, 1.0)


# revision 14
# speedup vs baseline: 2.0899x; 1.0538x over previous
"""GaussianImage (Cholesky) renderer on 8 trn2 NeuronCores.

Strategy: tile-parallel over the pixel grid.  The 256x256 image is cut
into 32x32-pixel tiles (64/frame, 128 total for T=2).  The host bins
gaussians to tiles (bbox intersect via a conservative support radius;
outside it exp(-sigma) underflows to 0 in fp32), pads each tile's
gaussian list to 128 slots, and precomputes per-slot quadratic
coefficients of sigma in local tile coordinates:

  sigma(gx,gy) = A gx^2 + B gx gy + C gy^2 + D gx + E gy + F

Each coefficient is split hi/lo into two fp16 values (lo pre-scaled by
2^11 to stay in fp16 normal range; the matching basis rows are scaled
by 2^-11), giving ~fp32-accurate sigma from a single K=12 fp16 matmul
that runs at 4x the fp32 rate on the PE array.  Per tile-entry:

  sigma = lhsT(12,128)^T @ basis(12,1024)      [TensorE fp16, fp32 PSUM]
  alpha = Exp(-sigma)                          [ScalarE -> fp16]
  img   = w(128,3)^T @ alpha(128,1024)         [TensorE fp16, fp32 PSUM]

Images for 4 consecutive entries land in one PSUM tile at partition
offsets 0/32/64/96 (PE column-group tiling) and are DMA'd out unclamped
in a single strided transfer; the final clip(0,1) runs on the host
during unsharding.  Each pixel is owned by exactly one tile -> no
cross-core reduction.
"""

import os
import numpy as np

T, N, H, W = 2, 512, 256, 256
TILE = 32
NT = H // TILE          # 8 tiles per axis
N_CORES = 8
SLOTS = 128
PIX = TILE * TILE       # 1024
SIGMA_CUT = 100.0       # exp(-100) ~ 4e-44: below fp32 denormal resolution
LO_SCALE = 2048.0       # 2^11: keeps lo-half fp16 coefficients normal

_CACHE = {}


def _build_nc(E):
    import concourse.bass as bass
    import concourse.mybir as mybir
    from concourse.tile import TileContext
    import bass_rust

    f32 = mybir.dt.float32
    f16 = mybir.dt.float16
    Act = mybir.ActivationFunctionType

    Alu = mybir.AluOpType
    G = E // 4  # img PSUM groups of 4 entries

    nc = bass.Bass("TRN2")
    # cb packs [basis (cols 0:1024) | per-entry lhsT coeffs (128 cols each)]
    CB = PIX + E * SLOTS
    cb = nc.dram_tensor("cb", [12, CB], f16, kind="ExternalInput")
    wtr = nc.dram_tensor("wtr", [SLOTS, E * 3], f16, kind="ExternalInput")
    out = nc.dram_tensor("out", [G, SLOTS, PIX], f16, kind="ExternalOutput")

    with TileContext(nc) as tc:
        with tc.tile_pool(name="const", bufs=1) as cpool, \
             tc.tile_pool(name="alpha", bufs=4) as apool, \
             tc.tile_pool(name="ps_sig", bufs=2, space="PSUM") as pss, \
             tc.tile_pool(name="ps_img", bufs=2, space="PSUM") as psi:

            S = nc.scalar

            # preload the Exp activation table while input DMAs are in
            # flight (the table load ~1.3us otherwise serializes with the
            # first real exp)
            warm = cpool.tile([SLOTS, 1], f32, tag="warm")
            nc.gpsimd.memset(warm, 0.0)
            S.activation(warm, warm, Act.Exp)

            ct = cpool.tile([12, CB], f16, tag="cb")
            wt = cpool.tile([SLOTS, E * 3], f16, tag="wtr")
            # split the coeff DMA so the first entries' matmuls only wait
            # for the first chunk; wtr goes out on the scalar engine's DMA
            # queue so it runs in parallel
            SPLIT = PIX + 4 * SLOTS
            nc.sync.dma_start(out=ct[:, 0:SPLIT], in_=cb[:, 0:SPLIT])
            nc.sync.dma_start(out=ct[:, SPLIT:CB], in_=cb[:, SPLIT:CB])
            nc.scalar.dma_start(out=wt, in_=wtr[:])
            bt = ct[:, 0:PIX]

            alphas = {}
            imgt = {}

            def emit_img(e):
                g, i = divmod(e, 4)
                if i == 0:
                    imgt[g] = psi.tile([SLOTS, PIX], f32, tag="img",
                                       name=f"img{g}")
                t = imgt[g]
                al = alphas.pop(e)
                wre = wt[:, 3 * e:3 * e + 3]
                nc.tensor.matmul(t[32 * i:32 * i + 3, 0:512], wre,
                                 al[:, 0:512], start=True, stop=True,
                                 tile_position=(0, 32 * i))
                nc.tensor.matmul(t[32 * i:32 * i + 3, 512:1024], wre,
                                 al[:, 512:1024], start=True, stop=True,
                                 tile_position=(0, 32 * i))
                if i == 3:
                    # fused clamp + PSUM->SBUF fp16 copy for the whole
                    # 4-entry group (one DVE op: cost is cols, not rows)
                    st = apool.tile([SLOTS, PIX], f16, tag="st",
                                    name=f"st{g}")
                    nc.vector.tensor_scalar(out=st, in0=t, scalar1=0.0,
                                            scalar2=1.0, op0=Alu.max,
                                            op1=Alu.min)
                    # NOTE: a two-level partition AP ("(a r) f -> a r f"
                    # then slicing r) mis-lowers in DMA codegen (only the
                    # first row of each 32-block transfers), so ship the
                    # whole clamped tile and slice on the host.
                    nc.sync.dma_start(out=out[g], in_=st)

            for e in range(E):
                sig = pss.tile([SLOTS, PIX], f32, tag="sig", name=f"sig{e}")
                lh = ct[:, PIX + SLOTS * e:PIX + SLOTS * (e + 1)]
                nc.tensor.matmul(sig[:, 0:512], lh, bt[:, 0:512],
                                 start=True, stop=True)
                nc.tensor.matmul(sig[:, 512:1024], lh, bt[:, 512:1024],
                                 start=True, stop=True)
                # software pipeline with a lag of 2 entries: by the time
                # the img matmuls of entry e-2 issue, its exp finished long
                # ago, so the PE never stalls on the ScalarE
                if e >= 2:
                    emit_img(e - 2)
                al = apool.tile([SLOTS, PIX], f16, tag="alpha", name=f"al{e}")
                S.activation(al, sig, Act.Exp, scale=-1.0)
                alphas[e] = al
            emit_img(E - 2)
            emit_img(E - 1)

    bass_rust.generate_event_semaphores(nc)
    return nc


def _bin_entries(cx, cy, lam):
    """Host-side routing: which gaussians overlap which 32x32 tile."""
    r = np.sqrt(2.0 * SIGMA_CUT * np.maximum(lam, 0.0)) + 1.0

    entries = []  # (frame, ty, tx, index-list)
    for t in range(T):
        x0 = np.clip(((cx[t] - r[t]) // TILE).astype(int), 0, NT - 1)
        x1 = np.clip(((cx[t] + r[t]) // TILE).astype(int), 0, NT - 1)
        y0 = np.clip(((cy[t] - r[t]) // TILE).astype(int), 0, NT - 1)
        y1 = np.clip(((cy[t] + r[t]) // TILE).astype(int), 0, NT - 1)
        buckets = [[[] for _ in range(NT)] for _ in range(NT)]
        for n in range(N):
            for ty in range(y0[n], y1[n] + 1):
                for tx in range(x0[n], x1[n] + 1):
                    buckets[ty][tx].append(n)
        for ty in range(NT):
            for tx in range(NT):
                assert len(buckets[ty][tx]) <= SLOTS, "tile overflow: >128 gaussians"
                entries.append((t, ty, tx, buckets[ty][tx]))
    return entries


def _ensure_ntff_hook():
    """Provide antenv.axon_hooks (missing in this image) so trace=True works."""
    import sys, types, ctypes, contextlib
    if "antenv.axon_hooks" in sys.modules:
        return
    so_path = "/opt/axon/libaxon_pjrt.so"
    if not os.path.exists(so_path):
        return
    lib = ctypes.CDLL(so_path)
    if not hasattr(lib, "axon_start_nrt_profile"):
        return
    lib.axon_start_nrt_profile.argtypes = [ctypes.POINTER(ctypes.c_int64), ctypes.c_size_t]
    lib.axon_start_nrt_profile.restype = ctypes.c_int64
    lib.axon_stop_nrt_profile.argtypes = [ctypes.c_char_p]
    lib.axon_stop_nrt_profile.restype = ctypes.c_int64

    @contextlib.contextmanager
    def _hook(output_dir, device_ids):
        import jax
        jax.devices()
        if device_ids:
            ids = (ctypes.c_int64 * len(device_ids))(*device_ids)
            rc = lib.axon_start_nrt_profile(ids, len(device_ids))
        else:
            rc = lib.axon_start_nrt_profile(None, 0)
        if rc != 0:
            raise RuntimeError(f"axon_start_nrt_profile rc={rc}")
        try:
            yield
        finally:
            n = lib.axon_stop_nrt_profile(str(output_dir).encode())
            print(f"profile: {n} file(s) written to {output_dir}")

    mod = types.ModuleType("antenv.axon_hooks")
    mod.get_axon_ntff_profile_hook = lambda: _hook
    mod.set_axon_ntff_profile_hook = lambda h: None
    sys.modules["antenv.axon_hooks"] = mod


def _split16(c):
    """Split float64 array c into (hi, lo) fp16 with lo pre-scaled by 2^11."""
    hi = c.astype(np.float16)
    lo = ((c - hi.astype(np.float64)) * LO_SCALE).astype(np.float16)
    return hi, lo


def kernel(xyz, cholesky, opacity, features_dc):
    from concourse import bass_utils

    xyz = np.asarray(xyz, np.float32)
    cholesky = np.asarray(cholesky, np.float32)
    opacity = np.asarray(opacity, np.float32)
    features_dc = np.asarray(features_dc, np.float32)

    # ---- host precompute (float64): projection, conic, binning ----
    means = np.tanh(xyz.astype(np.float64))
    cx = 0.5 * W * (means[..., 0] + 1.0)                    # (T,N)
    cy = 0.5 * H * (means[..., 1] + 1.0)
    chol = cholesky.astype(np.float64) + np.array([0.5, 0.0, 0.5])
    l0, l1, l2 = chol[..., 0], chol[..., 1], chol[..., 2]
    sxx, sxy, syy = l0 * l0, l0 * l1, l1 * l1 + l2 * l2
    det = sxx * syy - sxy * sxy
    ca, cb, cc = syy / det, -sxy / det, sxx / det           # conic (T,N)
    tr = sxx + syy
    lam = tr / 2 + np.sqrt(np.maximum(tr * tr / 4 - det, 0.0))

    colors = 1.0 / (1.0 + np.exp(-features_dc.astype(np.float64)))   # (N,3)
    opac = 1.0 / (1.0 + np.exp(-opacity.astype(np.float64)[:, 0]))   # (N,)
    w3 = colors * opac[:, None]                                      # (N,3)

    entries = _bin_entries(cx, cy, lam)
    E = (len(entries) + N_CORES - 1) // N_CORES

    # fp16 quadratic basis over local 32x32 pixels; rows 6-11 are the
    # lo-coefficient rows, scaled by 2^-11 (power of two: still exact)
    gx = np.arange(PIX, dtype=np.float64) % TILE
    gy = np.arange(PIX, dtype=np.float64) // TILE
    b6 = np.stack([gx * gx, gx * gy, gy * gy, gx, gy, np.ones(PIX)])
    basis = np.concatenate([b6, b6 / LO_SCALE]).astype(np.float16)

    in_maps = []
    for c in range(N_CORES):
        cbm = np.zeros((12, PIX + E * SLOTS), np.float16)
        cbm[:, 0:PIX] = basis
        lm = cbm[:, PIX:]
        wm = np.zeros((SLOTS, E * 3), np.float16)
        for ei in range(E):
            k = c * E + ei
            if k >= len(entries):
                continue
            t, ty, tx, idxs = entries[k]
            ns = len(idxs)
            if not ns:
                continue
            idxs = np.asarray(idxs)
            ex = cx[t, idxs] - tx * TILE
            ey = cy[t, idxs] - ty * TILE
            a_, b_, c_ = ca[t, idxs], cb[t, idxs], cc[t, idxs]
            coef = np.stack([
                0.5 * a_,
                b_,
                0.5 * c_,
                -(a_ * ex + b_ * ey),
                -(b_ * ex + c_ * ey),
                0.5 * (a_ * ex * ex + c_ * ey * ey) + b_ * ex * ey,
            ])                                               # (6, ns)
            hi, lo = _split16(coef)
            s = slice(SLOTS * ei, SLOTS * ei + ns)
            lm[0:6, s] = hi
            lm[6:12, s] = lo
            wm[:ns, 3 * ei:3 * ei + 3] = w3[idxs].astype(np.float16)
        in_maps.append({"cb": cbm, "wtr": wm})

    if E not in _CACHE:
        _CACHE[E] = _build_nc(E)
    nc = _CACHE[E]

    trace = bool(int(os.environ.get("GS_TRACE", "0")))
    if trace:
        _ensure_ntff_hook()
    res = bass_utils.run_bass_kernel_spmd(
        nc, in_maps, core_ids=list(range(N_CORES)), trace=trace)
    kernel.last_result = res

    img = np.zeros((T, 3, H, W), np.float32)
    for c in range(N_CORES):
        # out[g] is the full 128-partition clamp tile; entry i of group g
        # lives at partitions 32i..32i+2
        o = res.results[c]["out"].reshape(E // 4, 4, 32, PIX)[:, :, 0:3, :]
        o = o.reshape(E, 3, TILE, TILE)
        for ei in range(E):
            k = c * E + ei
            if k >= len(entries):
                continue
            t, ty, tx, _ = entries[k]
            img[t, :, ty * TILE:(ty + 1) * TILE, tx * TILE:(tx + 1) * TILE] = o[ei]
    return np.clip(img, 0.0, 1.0)


# revision 15
# speedup vs baseline: 3.4769x; 1.6637x over previous
"""GaussianImage (Cholesky) renderer on 8 trn2 NeuronCores.

Strategy: tile-parallel over the pixel grid with multi-tile slot packing.
The 256x256 image is cut into 32x32-pixel tiles (64/frame, 128 total for
T=2).  The host bins gaussians to tiles (bbox intersect via a
conservative support radius; outside it exp(-sigma) underflows to 0 in
fp32) and then bin-packs several tiles into one 128-slot "group"
(occupancies sum to <= 128; mean tile occupancy is ~40).  All tiles share
the same local 32x32 quadratic basis, so one K=12 fp16 matmul + one Exp
evaluates every gaussian of every tile in the group against all 1024
local pixels:

  sigma = lhsT(12,128)^T @ basis(12,1024)      [TensorE fp16, fp32 PSUM]
  alpha = Exp(-sigma)                          [ScalarE -> fp16]
  img   = w(128,32)^T @ alpha(128,1024)        [TensorE fp16, fp32 PSUM]

The img weights are block-structured: column 3j+c holds channel-c colors
of tile j's gaussians at their slots and zeros elsewhere, so one matmul
renders every tile of the group (row 3j+c = tile j, channel c).  Per-slot
sigma coefficients (quadratic in local pixel coords) are precomputed on
the host and split hi/lo into two fp16 values (lo pre-scaled by 2^11 to
stay in fp16 normal range; matching basis rows scaled by 2^-11), giving
~fp32-accurate sigma at the PE's full fp16 rate.  Images for 4 groups
land in one PSUM tile at partition offsets 0/32/64/96 (PE column-group
tiling), are clamped+converted to fp16 by one VectorE op, and DMA'd out.
Each pixel is owned by exactly one tile -> no cross-core reduction.
"""

import os
import numpy as np

T, N, H, W = 2, 512, 256, 256
TILE = 32
NT = H // TILE          # 8 tiles per axis
N_CORES = 8
SLOTS = 128
PIX = TILE * TILE       # 1024
MAXTILES = 10           # 3*MAXTILES <= 32 img-weight columns per group
SIGMA_CUT = 100.0       # exp(-100) ~ 4e-44: below fp32 denormal resolution
LO_SCALE = 2048.0       # 2^11: keeps lo-half fp16 coefficients normal

_CACHE = {}


def _build_nc(E):
    import concourse.bass as bass
    import concourse.mybir as mybir
    from concourse.tile import TileContext
    import bass_rust

    f32 = mybir.dt.float32
    f16 = mybir.dt.float16
    Act = mybir.ActivationFunctionType
    Alu = mybir.AluOpType
    G = (E + 3) // 4  # img PSUM tiles, 4 groups each

    nc = bass.Bass("TRN2")
    # cb packs [basis (cols 0:1024) | per-group lhsT coeffs (128 cols each)]
    CB = PIX + E * SLOTS
    cb = nc.dram_tensor("cb", [12, CB], f16, kind="ExternalInput")
    wtr = nc.dram_tensor("wtr", [SLOTS, E * 32], f16, kind="ExternalInput")
    out = nc.dram_tensor("out", [G, SLOTS, PIX], f16, kind="ExternalOutput")

    with TileContext(nc) as tc:
        with tc.tile_pool(name="const", bufs=1) as cpool, \
             tc.tile_pool(name="alpha", bufs=3) as apool, \
             tc.tile_pool(name="ps_sig", bufs=2, space="PSUM") as pss, \
             tc.tile_pool(name="ps_img", bufs=2, space="PSUM") as psi:

            S = nc.scalar

            # preload the Exp activation table while input DMAs are in
            # flight (the table load ~1.3us otherwise serializes with the
            # first real exp)
            warm = cpool.tile([SLOTS, 1], f32, tag="warm")
            nc.gpsimd.memset(warm, 0.0)
            S.activation(warm, warm, Act.Exp)

            ct = cpool.tile([12, CB], f16, tag="cb")
            wt = cpool.tile([SLOTS, E * 32], f16, tag="wtr")
            # split the coeff DMA so the first groups' matmuls only wait
            # for the first chunk; wtr goes out on the scalar engine's DMA
            # queue so it runs in parallel
            SPLIT = min(PIX + 2 * SLOTS, CB)
            nc.sync.dma_start(out=ct[:, 0:SPLIT], in_=cb[:, 0:SPLIT])
            if SPLIT < CB:
                nc.sync.dma_start(out=ct[:, SPLIT:CB], in_=cb[:, SPLIT:CB])
            nc.scalar.dma_start(out=wt, in_=wtr[:])
            bt = ct[:, 0:PIX]

            alphas = {}
            imgt = {}

            def emit_img(e):
                g, i = divmod(e, 4)
                if i == 0:
                    imgt[g] = psi.tile([SLOTS, PIX], f32, tag="img",
                                       name=f"img{g}")
                t = imgt[g]
                al = alphas.pop(e)
                wre = wt[:, 32 * e:32 * e + 32]
                nc.tensor.matmul(t[32 * i:32 * i + 32, 0:512], wre,
                                 al[:, 0:512], start=True, stop=True,
                                 tile_position=(0, 32 * i))
                nc.tensor.matmul(t[32 * i:32 * i + 32, 512:1024], wre,
                                 al[:, 512:1024], start=True, stop=True,
                                 tile_position=(0, 32 * i))
                if i == 3 or e == E - 1:
                    # fused clamp + PSUM->SBUF fp16 copy for the whole
                    # 4-group block (one DVE op: cost is cols, not rows)
                    st = apool.tile([SLOTS, PIX], f16, tag="st",
                                    name=f"st{g}")
                    nc.vector.tensor_scalar(out=st, in0=t, scalar1=0.0,
                                            scalar2=1.0, op0=Alu.max,
                                            op1=Alu.min)
                    nc.sync.dma_start(out=out[g], in_=st)

            for e in range(E):
                sig = pss.tile([SLOTS, PIX], f32, tag="sig", name=f"sig{e}")
                lh = ct[:, PIX + SLOTS * e:PIX + SLOTS * (e + 1)]
                nc.tensor.matmul(sig[:, 0:512], lh, bt[:, 0:512],
                                 start=True, stop=True)
                nc.tensor.matmul(sig[:, 512:1024], lh, bt[:, 512:1024],
                                 start=True, stop=True)
                # software pipeline: img matmuls of the previous group go
                # between this group's sig matmuls and its exp
                if e > 0:
                    emit_img(e - 1)
                al = apool.tile([SLOTS, PIX], f16, tag="alpha", name=f"al{e}")
                S.activation(al, sig, Act.Exp, scale=-1.0)
                alphas[e] = al
            emit_img(E - 1)

    bass_rust.generate_event_semaphores(nc)
    return nc


def _bin_entries(cx, cy, lam):
    """Host-side routing: which gaussians overlap which 32x32 tile."""
    r = np.sqrt(2.0 * SIGMA_CUT * np.maximum(lam, 0.0)) + 1.0

    entries = []  # (frame, ty, tx, index-list)
    for t in range(T):
        x0 = np.clip(((cx[t] - r[t]) // TILE).astype(int), 0, NT - 1)
        x1 = np.clip(((cx[t] + r[t]) // TILE).astype(int), 0, NT - 1)
        y0 = np.clip(((cy[t] - r[t]) // TILE).astype(int), 0, NT - 1)
        y1 = np.clip(((cy[t] + r[t]) // TILE).astype(int), 0, NT - 1)
        buckets = [[[] for _ in range(NT)] for _ in range(NT)]
        for n in range(N):
            for ty in range(y0[n], y1[n] + 1):
                for tx in range(x0[n], x1[n] + 1):
                    buckets[ty][tx].append(n)
        for ty in range(NT):
            for tx in range(NT):
                assert len(buckets[ty][tx]) <= SLOTS, "tile overflow: >128 gaussians"
                entries.append((t, ty, tx, buckets[ty][tx]))
    return entries


def _pack_bins(entries, bins_per_core):
    """Pack tile-entries into groups of <= 128 total slots, <= MAXTILES
    tiles, load-balanced over N_CORES * bins_per_core bins. Returns a list
    of bins, each a list of entry indices, or None if infeasible."""
    nbins = N_CORES * bins_per_core
    order = sorted(range(len(entries)),
                   key=lambda k: -len(entries[k][3]))
    loads = [0] * nbins
    counts = [0] * nbins
    bins = [[] for _ in range(nbins)]
    for k in order:
        occ = len(entries[k][3])
        best = -1
        for b in sorted(range(nbins), key=lambda b: loads[b]):
            if counts[b] < MAXTILES and loads[b] + occ <= SLOTS:
                best = b
                break
        if best < 0:
            return None
        bins[best].append(k)
        loads[best] += occ
        counts[best] += 1
    return bins


def _ensure_ntff_hook():
    """Provide antenv.axon_hooks (missing in this image) so trace=True works."""
    import sys, types, ctypes, contextlib
    if "antenv.axon_hooks" in sys.modules:
        return
    so_path = "/opt/axon/libaxon_pjrt.so"
    if not os.path.exists(so_path):
        return
    lib = ctypes.CDLL(so_path)
    if not hasattr(lib, "axon_start_nrt_profile"):
        return
    lib.axon_start_nrt_profile.argtypes = [ctypes.POINTER(ctypes.c_int64), ctypes.c_size_t]
    lib.axon_start_nrt_profile.restype = ctypes.c_int64
    lib.axon_stop_nrt_profile.argtypes = [ctypes.c_char_p]
    lib.axon_stop_nrt_profile.restype = ctypes.c_int64

    @contextlib.contextmanager
    def _hook(output_dir, device_ids):
        import jax
        jax.devices()
        if device_ids:
            ids = (ctypes.c_int64 * len(device_ids))(*device_ids)
            rc = lib.axon_start_nrt_profile(ids, len(device_ids))
        else:
            rc = lib.axon_start_nrt_profile(None, 0)
        if rc != 0:
            raise RuntimeError(f"axon_start_nrt_profile rc={rc}")
        try:
            yield
        finally:
            n = lib.axon_stop_nrt_profile(str(output_dir).encode())
            print(f"profile: {n} file(s) written to {output_dir}")

    mod = types.ModuleType("antenv.axon_hooks")
    mod.get_axon_ntff_profile_hook = lambda: _hook
    mod.set_axon_ntff_profile_hook = lambda h: None
    sys.modules["antenv.axon_hooks"] = mod


def _split16(c):
    """Split float64 array c into (hi, lo) fp16 with lo pre-scaled by 2^11."""
    hi = c.astype(np.float16)
    lo = ((c - hi.astype(np.float64)) * LO_SCALE).astype(np.float16)
    return hi, lo


def kernel(xyz, cholesky, opacity, features_dc):
    from concourse import bass_utils

    xyz = np.asarray(xyz, np.float32)
    cholesky = np.asarray(cholesky, np.float32)
    opacity = np.asarray(opacity, np.float32)
    features_dc = np.asarray(features_dc, np.float32)

    # ---- host precompute (float64): projection, conic, binning ----
    means = np.tanh(xyz.astype(np.float64))
    cx = 0.5 * W * (means[..., 0] + 1.0)                    # (T,N)
    cy = 0.5 * H * (means[..., 1] + 1.0)
    chol = cholesky.astype(np.float64) + np.array([0.5, 0.0, 0.5])
    l0, l1, l2 = chol[..., 0], chol[..., 1], chol[..., 2]
    sxx, sxy, syy = l0 * l0, l0 * l1, l1 * l1 + l2 * l2
    det = sxx * syy - sxy * sxy
    ca, cb, cc = syy / det, -sxy / det, sxx / det           # conic (T,N)
    tr = sxx + syy
    lam = tr / 2 + np.sqrt(np.maximum(tr * tr / 4 - det, 0.0))

    colors = 1.0 / (1.0 + np.exp(-features_dc.astype(np.float64)))   # (N,3)
    opac = 1.0 / (1.0 + np.exp(-opacity.astype(np.float64)[:, 0]))   # (N,)
    w3 = colors * opac[:, None]                                      # (N,3)

    entries = _bin_entries(cx, cy, lam)
    total = sum(len(e[3]) for e in entries)
    E = max(2, -(-total // (SLOTS * N_CORES)))   # bins per core, lower bound
    bins = None
    while bins is None:
        bins = _pack_bins(entries, E)
        if bins is None:
            E += 1

    # fp16 quadratic basis over local 32x32 pixels; rows 6-11 are the
    # lo-coefficient rows, scaled by 2^-11 (power of two: still exact)
    gx = np.arange(PIX, dtype=np.float64) % TILE
    gy = np.arange(PIX, dtype=np.float64) // TILE
    b6 = np.stack([gx * gx, gx * gy, gy * gy, gx, gy, np.ones(PIX)])
    basis = np.concatenate([b6, b6 / LO_SCALE]).astype(np.float16)

    in_maps = []
    for c in range(N_CORES):
        cbm = np.zeros((12, PIX + E * SLOTS), np.float16)
        cbm[:, 0:PIX] = basis
        lm = cbm[:, PIX:]
        wm = np.zeros((SLOTS, E * 32), np.float16)
        for ei in range(E):
            off = 0
            for j, k in enumerate(bins[c * E + ei]):
                t, ty, tx, idxs = entries[k]
                ns = len(idxs)
                if not ns:
                    continue
                idxs = np.asarray(idxs)
                ex = cx[t, idxs] - tx * TILE
                ey = cy[t, idxs] - ty * TILE
                a_, b_, c_ = ca[t, idxs], cb[t, idxs], cc[t, idxs]
                coef = np.stack([
                    0.5 * a_,
                    b_,
                    0.5 * c_,
                    -(a_ * ex + b_ * ey),
                    -(b_ * ex + c_ * ey),
                    0.5 * (a_ * ex * ex + c_ * ey * ey) + b_ * ex * ey,
                ])                                           # (6, ns)
                hi, lo = _split16(coef)
                s = slice(SLOTS * ei + off, SLOTS * ei + off + ns)
                lm[0:6, s] = hi
                lm[6:12, s] = lo
                wm[off:off + ns, 32 * ei + 3 * j:32 * ei + 3 * j + 3] = \
                    w3[idxs].astype(np.float16)
                off += ns
        in_maps.append({"cb": cbm, "wtr": wm})

    if E not in _CACHE:
        _CACHE[E] = _build_nc(E)
    nc = _CACHE[E]

    trace = bool(int(os.environ.get("GS_TRACE", "0")))
    if trace:
        _ensure_ntff_hook()
    res = bass_utils.run_bass_kernel_spmd(
        nc, in_maps, core_ids=list(range(N_CORES)), trace=trace)
    kernel.last_result = res

    img = np.zeros((T, 3, H, W), np.float32)
    for c in range(N_CORES):
        o = res.results[c]["out"]     # (G, 128, PIX) fp16
        for ei in range(E):
            g, i = divmod(ei, 4)
            for j, k in enumerate(bins[c * E + ei]):
                t, ty, tx, _ = entries[k]
                blk = o[g, 32 * i + 3 * j:32 * i + 3 * j + 3]
                img[t, :, ty * TILE:(ty + 1) * TILE,
                    tx * TILE:(tx + 1) * TILE] = \
                    blk.reshape(3, TILE, TILE)
    return np.clip(img, 0.0, 1.0)


# revision 21
# speedup vs baseline: 4.5892x; 1.3199x over previous
"""GaussianImage (Cholesky) renderer on 8 trn2 NeuronCores.

Strategy: tile-parallel over the pixel grid with multi-tile slot packing.
The 256x256 image is cut into 32x32-pixel tiles (64/frame, 128 total for
T=2).  The host bins gaussians to tiles (bbox intersect via a
conservative support radius; outside it exp(-sigma) underflows to 0 in
fp32) and then bin-packs several tiles into one 128-slot "group"
(occupancies sum to <= 128; mean tile occupancy is ~40).  All tiles share
the same local 32x32 quadratic basis, so one K=12 fp16 matmul + one Exp
evaluates every gaussian of every tile in the group against all 1024
local pixels:

  sigma = lhsT(12,128)^T @ basis(12,1024)      [TensorE fp16, fp32 PSUM]
  alpha = Exp(-sigma)                          [ScalarE -> fp16]
  img   = w(128,32)^T @ alpha(128,1024)        [TensorE fp16, fp32 PSUM]

The img weights are block-structured: column 3j+c holds channel-c colors
of tile j's gaussians at their slots and zeros elsewhere, so one matmul
renders every tile of the group (row 3j+c = tile j, channel c).  Per-slot
sigma coefficients (quadratic in local pixel coords) are precomputed on
the host and split hi/lo into two fp16 values (lo pre-scaled by 2^11 to
stay in fp16 normal range; matching basis rows scaled by 2^-11), giving
~fp32-accurate sigma at the PE's full fp16 rate.  Images for 4 groups
land in one PSUM tile at partition offsets 0/32/64/96 (PE column-group
tiling), are clamped+converted to fp16 by one VectorE op, and DMA'd out.
Each pixel is owned by exactly one tile -> no cross-core reduction.
"""

import os
import numpy as np

T, N, H, W = 2, 512, 256, 256
TILE = 32
NT = H // TILE          # 8 tiles per axis
N_CORES = 8
SLOTS = 128
PIX = TILE * TILE       # 1024
MAXTILES = 10           # 3*MAXTILES <= 32 img-weight columns per group
SIGMA_CUT = 20.0        # exp(-20) ~ 2e-9: negligible vs the 2e-2 rel gate
LO_SCALE = 2048.0       # 2^11: keeps lo-half fp16 coefficients normal

_CACHE = {}


def _build_nc(E):
    import concourse.bass as bass
    import concourse.mybir as mybir
    from concourse.tile import TileContext
    import bass_rust

    f32 = mybir.dt.float32
    f16 = mybir.dt.float16
    Act = mybir.ActivationFunctionType
    Alu = mybir.AluOpType
    G = (E + 3) // 4  # img PSUM tiles, 4 groups each

    nc = bass.Bass("TRN2")
    # cba packs [basis | group-0 lhsT coeffs]; cbb has the remaining
    # groups' coeffs.  Separate tensors/tiles so the first sig matmul only
    # depends on the small first DMA.
    cba = nc.dram_tensor("cba", [12, PIX + SLOTS], f16, kind="ExternalInput")
    cbb = nc.dram_tensor("cbb", [12, max(E - 1, 1) * SLOTS], f16,
                         kind="ExternalInput")
    wtr = nc.dram_tensor("wtr", [SLOTS, E * 32], f16, kind="ExternalInput")
    out = nc.dram_tensor("out", [G, SLOTS, PIX], f16, kind="ExternalOutput")

    with TileContext(nc) as tc:
        with tc.tile_pool(name="const", bufs=1) as cpool, \
             tc.tile_pool(name="alpha", bufs=4) as apool, \
             tc.tile_pool(name="ps_sig", bufs=2, space="PSUM") as pss, \
             tc.tile_pool(name="ps_img", bufs=2 if G > 1 else 1,
                          space="PSUM") as psi:

            S = nc.scalar

            cta = cpool.tile([12, PIX + SLOTS], f16, tag="cba")
            ctb = cpool.tile([12, max(E - 1, 1) * SLOTS], f16, tag="cbb")
            wt = cpool.tile([SLOTS, E * 32], f16, tag="wtr")
            # wtr rides the scalar engine's DMA queue, in parallel with the
            # sync-queue coeff DMAs
            nc.scalar.dma_start(out=wt, in_=wtr[:])
            nc.sync.dma_start(out=cta, in_=cba[:])
            nc.sync.dma_start(out=ctb, in_=cbb[:])

            # preload the Exp activation table while input DMAs are in
            # flight (the table load ~1.3us otherwise serializes with the
            # first real exp)
            warm = cpool.tile([SLOTS, 1], f32, tag="warm")
            nc.gpsimd.memset(warm, 0.0)
            S.activation(warm, warm, Act.Exp)

            bt = cta[:, 0:PIX]

            alphas = {}
            imgt = {}

            def emit_img(e):
                g, i = divmod(e, 4)
                if i == 0:
                    imgt[g] = psi.tile([SLOTS, PIX], f32, tag="img",
                                       name=f"img{g}")
                t = imgt[g]
                al = alphas.pop(e)
                wre = wt[:, 32 * e:32 * e + 32]
                final = e == E - 1
                if final:
                    st = apool.tile([SLOTS, PIX], f16, tag="st",
                                    name=f"st{g}")
                for (c0, c1) in ((0, 512), (512, 1024)):
                    nc.tensor.matmul(t[32 * i:32 * i + 32, c0:c1], wre,
                                     al[:, c0:c1], start=True, stop=True,
                                     tile_position=(0, 32 * i))
                    if final:
                        # drain the last PSUM tile in halves so the clamp
                        # and out-DMA overlap the second img matmul
                        nc.vector.tensor_scalar(out=st[:, c0:c1],
                                                in0=t[:, c0:c1],
                                                scalar1=0.0, scalar2=1.0,
                                                op0=Alu.max, op1=Alu.min)
                        nc.sync.dma_start(out=out[g][:, c0:c1],
                                          in_=st[:, c0:c1])
                if not final and i == 3:
                    # fused clamp + PSUM->SBUF fp16 copy for the whole
                    # 4-group block (one DVE op: cost is cols, not rows)
                    st = apool.tile([SLOTS, PIX], f16, tag="st",
                                    name=f"st{g}")
                    nc.vector.tensor_scalar(out=st, in0=t, scalar1=0.0,
                                            scalar2=1.0, op0=Alu.max,
                                            op1=Alu.min)
                    nc.sync.dma_start(out=out[g], in_=st)

            for e in range(E):
                sig = pss.tile([SLOTS, PIX], f32, tag="sig", name=f"sig{e}")
                lh = cta[:, PIX:PIX + SLOTS] if e == 0 else \
                    ctb[:, SLOTS * (e - 1):SLOTS * e]
                nc.tensor.matmul(sig[:, 0:512], lh, bt[:, 0:512],
                                 start=True, stop=True)
                nc.tensor.matmul(sig[:, 512:1024], lh, bt[:, 512:1024],
                                 start=True, stop=True)
                # software pipeline with a lag of 2 groups: by the time the
                # img matmuls of group e-2 issue, its exp finished long ago,
                # so the PE never stalls on the ScalarE
                if e >= 2:
                    emit_img(e - 2)
                al = apool.tile([SLOTS, PIX], f16, tag="alpha", name=f"al{e}")
                S.activation(al, sig, Act.Exp, scale=-1.0)
                alphas[e] = al
            if E >= 2:
                emit_img(E - 2)
            emit_img(E - 1)

    bass_rust.generate_event_semaphores(nc)
    return nc


def _bin_entries(cx, cy, lam):
    """Host-side routing: which gaussians overlap which 32x32 tile."""
    r = np.sqrt(2.0 * SIGMA_CUT * np.maximum(lam, 0.0)) + 1.0

    entries = []  # (frame, ty, tx, index-list)
    for t in range(T):
        x0 = np.clip(((cx[t] - r[t]) // TILE).astype(int), 0, NT - 1)
        x1 = np.clip(((cx[t] + r[t]) // TILE).astype(int), 0, NT - 1)
        y0 = np.clip(((cy[t] - r[t]) // TILE).astype(int), 0, NT - 1)
        y1 = np.clip(((cy[t] + r[t]) // TILE).astype(int), 0, NT - 1)
        buckets = [[[] for _ in range(NT)] for _ in range(NT)]
        for n in range(N):
            for ty in range(y0[n], y1[n] + 1):
                for tx in range(x0[n], x1[n] + 1):
                    buckets[ty][tx].append(n)
        for ty in range(NT):
            for tx in range(NT):
                assert len(buckets[ty][tx]) <= SLOTS, "tile overflow: >128 gaussians"
                entries.append((t, ty, tx, buckets[ty][tx]))
    return entries


def _pack_bins(entries, bins_per_core):
    """Pack tile-entries into groups of <= 128 total slots, <= MAXTILES
    tiles, load-balanced over N_CORES * bins_per_core bins. Returns a list
    of bins, each a list of entry indices, or None if infeasible."""
    nbins = N_CORES * bins_per_core
    order = sorted(range(len(entries)),
                   key=lambda k: -len(entries[k][3]))
    loads = [0] * nbins
    counts = [0] * nbins
    bins = [[] for _ in range(nbins)]
    for k in order:
        occ = len(entries[k][3])
        best = -1
        for b in sorted(range(nbins), key=lambda b: loads[b]):
            if counts[b] < MAXTILES and loads[b] + occ <= SLOTS:
                best = b
                break
        if best < 0:
            return None
        bins[best].append(k)
        loads[best] += occ
        counts[best] += 1
    return bins


def _ensure_ntff_hook():
    """Provide antenv.axon_hooks (missing in this image) so trace=True works."""
    import sys, types, ctypes, contextlib
    if "antenv.axon_hooks" in sys.modules:
        return
    so_path = "/opt/axon/libaxon_pjrt.so"
    if not os.path.exists(so_path):
        return
    lib = ctypes.CDLL(so_path)
    if not hasattr(lib, "axon_start_nrt_profile"):
        return
    lib.axon_start_nrt_profile.argtypes = [ctypes.POINTER(ctypes.c_int64), ctypes.c_size_t]
    lib.axon_start_nrt_profile.restype = ctypes.c_int64
    lib.axon_stop_nrt_profile.argtypes = [ctypes.c_char_p]
    lib.axon_stop_nrt_profile.restype = ctypes.c_int64

    @contextlib.contextmanager
    def _hook(output_dir, device_ids):
        import jax
        jax.devices()
        if device_ids:
            ids = (ctypes.c_int64 * len(device_ids))(*device_ids)
            rc = lib.axon_start_nrt_profile(ids, len(device_ids))
        else:
            rc = lib.axon_start_nrt_profile(None, 0)
        if rc != 0:
            raise RuntimeError(f"axon_start_nrt_profile rc={rc}")
        try:
            yield
        finally:
            n = lib.axon_stop_nrt_profile(str(output_dir).encode())
            print(f"profile: {n} file(s) written to {output_dir}")

    mod = types.ModuleType("antenv.axon_hooks")
    mod.get_axon_ntff_profile_hook = lambda: _hook
    mod.set_axon_ntff_profile_hook = lambda h: None
    sys.modules["antenv.axon_hooks"] = mod


def _split16(c):
    """Split float64 array c into (hi, lo) fp16 with lo pre-scaled by 2^11."""
    hi = c.astype(np.float16)
    lo = ((c - hi.astype(np.float64)) * LO_SCALE).astype(np.float16)
    return hi, lo


def kernel(xyz, cholesky, opacity, features_dc):
    from concourse import bass_utils

    xyz = np.asarray(xyz, np.float32)
    cholesky = np.asarray(cholesky, np.float32)
    opacity = np.asarray(opacity, np.float32)
    features_dc = np.asarray(features_dc, np.float32)

    # ---- host precompute (float64): projection, conic, binning ----
    means = np.tanh(xyz.astype(np.float64))
    cx = 0.5 * W * (means[..., 0] + 1.0)                    # (T,N)
    cy = 0.5 * H * (means[..., 1] + 1.0)
    chol = cholesky.astype(np.float64) + np.array([0.5, 0.0, 0.5])
    l0, l1, l2 = chol[..., 0], chol[..., 1], chol[..., 2]
    sxx, sxy, syy = l0 * l0, l0 * l1, l1 * l1 + l2 * l2
    det = sxx * syy - sxy * sxy
    ca, cb, cc = syy / det, -sxy / det, sxx / det           # conic (T,N)
    tr = sxx + syy
    lam = tr / 2 + np.sqrt(np.maximum(tr * tr / 4 - det, 0.0))

    colors = 1.0 / (1.0 + np.exp(-features_dc.astype(np.float64)))   # (N,3)
    opac = 1.0 / (1.0 + np.exp(-opacity.astype(np.float64)[:, 0]))   # (N,)
    w3 = colors * opac[:, None]                                      # (N,3)

    entries = _bin_entries(cx, cy, lam)
    total = sum(len(e[3]) for e in entries)
    E = max(2, -(-total // (SLOTS * N_CORES)))   # bins per core, lower bound
    bins = None
    while bins is None:
        bins = _pack_bins(entries, E)
        if bins is None:
            E += 1

    # fp16 quadratic basis over local 32x32 pixels; rows 6-11 are the
    # lo-coefficient rows, scaled by 2^-11 (power of two: still exact)
    gx = np.arange(PIX, dtype=np.float64) % TILE
    gy = np.arange(PIX, dtype=np.float64) // TILE
    b6 = np.stack([gx * gx, gx * gy, gy * gy, gx, gy, np.ones(PIX)])
    basis = np.concatenate([b6, b6 / LO_SCALE]).astype(np.float16)

    in_maps = []
    for c in range(N_CORES):
        cbam = np.zeros((12, PIX + SLOTS), np.float16)
        cbam[:, 0:PIX] = basis
        cbbm = np.zeros((12, max(E - 1, 1) * SLOTS), np.float16)
        lm = np.zeros((12, E * SLOTS), np.float16)
        wm = np.zeros((SLOTS, E * 32), np.float16)
        for ei in range(E):
            off = 0
            for j, k in enumerate(bins[c * E + ei]):
                t, ty, tx, idxs = entries[k]
                ns = len(idxs)
                if not ns:
                    continue
                idxs = np.asarray(idxs)
                ex = cx[t, idxs] - tx * TILE
                ey = cy[t, idxs] - ty * TILE
                a_, b_, c_ = ca[t, idxs], cb[t, idxs], cc[t, idxs]
                coef = np.stack([
                    0.5 * a_,
                    b_,
                    0.5 * c_,
                    -(a_ * ex + b_ * ey),
                    -(b_ * ex + c_ * ey),
                    0.5 * (a_ * ex * ex + c_ * ey * ey) + b_ * ex * ey,
                ])                                           # (6, ns)
                hi, lo = _split16(coef)
                s = slice(SLOTS * ei + off, SLOTS * ei + off + ns)
                lm[0:6, s] = hi
                lm[6:12, s] = lo
                wm[off:off + ns, 32 * ei + 3 * j:32 * ei + 3 * j + 3] = \
                    w3[idxs].astype(np.float16)
                off += ns
        cbam[:, PIX:] = lm[:, 0:SLOTS]
        if E > 1:
            cbbm[:] = lm[:, SLOTS:]
        in_maps.append({"cba": cbam, "cbb": cbbm, "wtr": wm})

    if E not in _CACHE:
        _CACHE[E] = _build_nc(E)
    nc = _CACHE[E]

    trace = bool(int(os.environ.get("GS_TRACE", "0")))
    if trace:
        _ensure_ntff_hook()
    res = bass_utils.run_bass_kernel_spmd(
        nc, in_maps, core_ids=list(range(N_CORES)), trace=trace)
    kernel.last_result = res

    img = np.zeros((T, 3, H, W), np.float32)
    for c in range(N_CORES):
        o = res.results[c]["out"]     # (G, 128, PIX) fp16
        for ei in range(E):
            g, i = divmod(ei, 4)
            for j, k in enumerate(bins[c * E + ei]):
                t, ty, tx, _ = entries[k]
                blk = o[g, 32 * i + 3 * j:32 * i + 3 * j + 3]
                img[t, :, ty * TILE:(ty + 1) * TILE,
                    tx * TILE:(tx + 1) * TILE] = \
                    blk.reshape(3, TILE, TILE)
    return np.clip(img, 0.0, 1.0)


# revision 26
# speedup vs baseline: 4.7719x; 1.0398x over previous
"""GaussianImage (Cholesky) renderer on 8 trn2 NeuronCores.

Strategy: tile-parallel over the pixel grid with multi-tile slot packing.
The 256x256 image is cut into 32x32-pixel tiles (64/frame, 128 total for
T=2).  The host bins gaussians to tiles (bbox intersect via a
conservative support radius; outside it exp(-sigma) underflows to 0 in
fp32) and then bin-packs several tiles into one 128-slot "group"
(occupancies sum to <= 128; mean tile occupancy is ~40).  All tiles share
the same local 32x32 quadratic basis, so one K=12 fp16 matmul + one Exp
evaluates every gaussian of every tile in the group against all 1024
local pixels:

  sigma = lhsT(12,128)^T @ basis(12,1024)      [TensorE fp16, fp32 PSUM]
  alpha = Exp(-sigma)                          [ScalarE -> fp16]
  img   = w(128,32)^T @ alpha(128,1024)        [TensorE fp16, fp32 PSUM]

The img weights are block-structured: column 3j+c holds channel-c colors
of tile j's gaussians at their slots and zeros elsewhere, so one matmul
renders every tile of the group (row 3j+c = tile j, channel c).  Per-slot
sigma coefficients (quadratic in local pixel coords) are precomputed on
the host and split hi/lo into two fp16 values (lo pre-scaled by 2^11 to
stay in fp16 normal range; matching basis rows scaled by 2^-11), giving
~fp32-accurate sigma at the PE's full fp16 rate.  Images for 4 groups
land in one PSUM tile at partition offsets 0/32/64/96 (PE column-group
tiling), are clamped+converted to fp16 by one VectorE op, and DMA'd out.
Each pixel is owned by exactly one tile -> no cross-core reduction.
"""

import os
import numpy as np

T, N, H, W = 2, 512, 256, 256
TILE = 32
NT = H // TILE          # 8 tiles per axis
N_CORES = 8
SLOTS = 128
PIX = TILE * TILE       # 1024
MAXTILES = 10           # 3*MAXTILES <= 32 img-weight columns per group
SIGMA_CUT = 20.0        # exp(-20) ~ 2e-9: negligible vs the 2e-2 rel gate
LO_SCALE = 2048.0       # 2^11: keeps lo-half fp16 coefficients normal

_CACHE = {}


def _build_nc(E):
    import concourse.bass as bass
    import concourse.mybir as mybir
    from concourse.tile import TileContext
    import bass_rust

    f32 = mybir.dt.float32
    f16 = mybir.dt.float16
    Act = mybir.ActivationFunctionType
    Alu = mybir.AluOpType
    G = (E + 3) // 4  # img PSUM tiles, 4 groups each

    nc = bass.Bass("TRN2")
    # Inputs split into four tensors so each rides a different engine's
    # DMA queue and the transfers run in parallel right after engine init:
    #   cba: [basis cols 0:512 | group-0 lhsT coeffs]   (sync queue)
    #   cbs: [basis cols 512:1024]                      (tensor queue)
    #   cbb: remaining groups' lhsT coeffs              (vector queue)
    #   wtr: img weights                                (scalar queue)
    cba = nc.dram_tensor("cba", [12, 512 + SLOTS], f16, kind="ExternalInput")
    cbs = nc.dram_tensor("cbs", [12, 512], f16, kind="ExternalInput")
    cbb = nc.dram_tensor("cbb", [12, max(E - 1, 1) * SLOTS], f16,
                         kind="ExternalInput")
    wtr = nc.dram_tensor("wtr", [SLOTS, E * 32], f16, kind="ExternalInput")
    out = nc.dram_tensor("out", [G, SLOTS, PIX], f16, kind="ExternalOutput")

    with TileContext(nc) as tc:
        with tc.tile_pool(name="const", bufs=1) as cpool, \
             tc.tile_pool(name="alpha", bufs=4) as apool, \
             tc.tile_pool(name="ps_sig", bufs=3 if G == 1 else 2,
                          space="PSUM") as pss, \
             tc.tile_pool(name="ps_img", bufs=2 if G > 1 else 1,
                          space="PSUM") as psi:

            S = nc.scalar

            # preload the Exp activation table first thing on the scalar
            # queue (the ~1.3us table load otherwise serializes with the
            # first real exp).  The warm tile is read uninitialized on
            # purpose: its output is never consumed.
            warm = cpool.tile([SLOTS, 1], f32, tag="warm")
            S.activation(warm, warm, Act.Exp)

            cta = cpool.tile([12, 512 + SLOTS], f16, tag="cba")
            cts = cpool.tile([12, 512], f16, tag="cbs")
            ctb = cpool.tile([12, max(E - 1, 1) * SLOTS], f16, tag="cbb")
            wt = cpool.tile([SLOTS, E * 32], f16, tag="wtr")
            nc.scalar.dma_start(out=wt, in_=wtr[:])
            nc.sync.dma_start(out=cta, in_=cba[:])
            nc.gpsimd.dma_start(out=cts, in_=cbs[:])
            nc.sync.dma_start(out=ctb, in_=cbb[:])

            def bt(c0, c1):
                return cta[:, c0:c1] if c1 <= 512 else cts[:, c0 - 512:c1 - 512]

            alphas = {}
            imgt = {}

            def emit_img(e):
                g, i = divmod(e, 4)
                if i == 0:
                    imgt[g] = psi.tile([SLOTS, PIX], f32, tag="img",
                                       name=f"img{g}")
                t = imgt[g]
                al = alphas.pop(e)
                wre = wt[:, 32 * e:32 * e + 32]
                final = e == E - 1
                if final:
                    st = apool.tile([SLOTS, PIX], f16, tag="st",
                                    name=f"st{g}")
                for (c0, c1) in ((0, 512), (512, 1024)):
                    nc.tensor.matmul(t[32 * i:32 * i + 32, c0:c1], wre,
                                     al[:, c0:c1], start=True, stop=True,
                                     tile_position=(0, 32 * i))
                    if final:
                        # drain the last PSUM tile in halves so the clamp
                        # and out-DMA overlap the second img matmul
                        nc.vector.tensor_scalar(out=st[:, c0:c1],
                                                in0=t[:, c0:c1],
                                                scalar1=0.0, scalar2=1.0,
                                                op0=Alu.max, op1=Alu.min)
                        nc.sync.dma_start(out=out[g][:, c0:c1],
                                          in_=st[:, c0:c1])
                if not final and i == 3:
                    # fused clamp + PSUM->SBUF fp16 copy for the whole
                    # 4-group block (one DVE op: cost is cols, not rows)
                    st = apool.tile([SLOTS, PIX], f16, tag="st",
                                    name=f"st{g}")
                    nc.vector.tensor_scalar(out=st, in0=t, scalar1=0.0,
                                            scalar2=1.0, op0=Alu.max,
                                            op1=Alu.min)
                    nc.sync.dma_start(out=out[g], in_=st)

            for e in range(E):
                sig = pss.tile([SLOTS, PIX], f32, tag="sig", name=f"sig{e}")
                lh = cta[:, 512:512 + SLOTS] if e == 0 else \
                    ctb[:, SLOTS * (e - 1):SLOTS * e]
                nc.tensor.matmul(sig[:, 0:512], lh, bt(0, 512),
                                 start=True, stop=True)
                nc.tensor.matmul(sig[:, 512:1024], lh, bt(512, 1024),
                                 start=True, stop=True)
                # software pipeline with a lag of 2 groups: by the time the
                # img matmuls of group e-2 issue, its exp finished long ago,
                # so the PE never stalls on the ScalarE
                if e >= 2:
                    emit_img(e - 2)
                al = apool.tile([SLOTS, PIX], f16, tag="alpha", name=f"al{e}")
                if e == E - 1:
                    # split the final exp so the drain chain
                    # sig->exp->img->clamp->dma pipelines at 512-col grain
                    S.activation(al[:, 0:512], sig[:, 0:512], Act.Exp,
                                 scale=-1.0)
                    S.activation(al[:, 512:1024], sig[:, 512:1024], Act.Exp,
                                 scale=-1.0)
                else:
                    S.activation(al, sig, Act.Exp, scale=-1.0)
                alphas[e] = al
            if E >= 2:
                emit_img(E - 2)
            emit_img(E - 1)

    bass_rust.generate_event_semaphores(nc)
    return nc


def _bin_entries(cx, cy, lam):
    """Host-side routing: which gaussians overlap which 32x32 tile."""
    r = np.sqrt(2.0 * SIGMA_CUT * np.maximum(lam, 0.0)) + 1.0

    entries = []  # (frame, ty, tx, index-list)
    for t in range(T):
        x0 = np.clip(((cx[t] - r[t]) // TILE).astype(int), 0, NT - 1)
        x1 = np.clip(((cx[t] + r[t]) // TILE).astype(int), 0, NT - 1)
        y0 = np.clip(((cy[t] - r[t]) // TILE).astype(int), 0, NT - 1)
        y1 = np.clip(((cy[t] + r[t]) // TILE).astype(int), 0, NT - 1)
        buckets = [[[] for _ in range(NT)] for _ in range(NT)]
        for n in range(N):
            for ty in range(y0[n], y1[n] + 1):
                for tx in range(x0[n], x1[n] + 1):
                    buckets[ty][tx].append(n)
        for ty in range(NT):
            for tx in range(NT):
                assert len(buckets[ty][tx]) <= SLOTS, "tile overflow: >128 gaussians"
                entries.append((t, ty, tx, buckets[ty][tx]))
    return entries


def _pack_bins(entries, bins_per_core):
    """Pack tile-entries into groups of <= 128 total slots, <= MAXTILES
    tiles, load-balanced over N_CORES * bins_per_core bins. Returns a list
    of bins, each a list of entry indices, or None if infeasible."""
    nbins = N_CORES * bins_per_core
    order = sorted(range(len(entries)),
                   key=lambda k: -len(entries[k][3]))
    loads = [0] * nbins
    counts = [0] * nbins
    bins = [[] for _ in range(nbins)]
    for k in order:
        occ = len(entries[k][3])
        best = -1
        for b in sorted(range(nbins), key=lambda b: loads[b]):
            if counts[b] < MAXTILES and loads[b] + occ <= SLOTS:
                best = b
                break
        if best < 0:
            return None
        bins[best].append(k)
        loads[best] += occ
        counts[best] += 1
    return bins


def _ensure_ntff_hook():
    """Provide antenv.axon_hooks (missing in this image) so trace=True works."""
    import sys, types, ctypes, contextlib
    if "antenv.axon_hooks" in sys.modules:
        return
    so_path = "/opt/axon/libaxon_pjrt.so"
    if not os.path.exists(so_path):
        return
    lib = ctypes.CDLL(so_path)
    if not hasattr(lib, "axon_start_nrt_profile"):
        return
    lib.axon_start_nrt_profile.argtypes = [ctypes.POINTER(ctypes.c_int64), ctypes.c_size_t]
    lib.axon_start_nrt_profile.restype = ctypes.c_int64
    lib.axon_stop_nrt_profile.argtypes = [ctypes.c_char_p]
    lib.axon_stop_nrt_profile.restype = ctypes.c_int64

    @contextlib.contextmanager
    def _hook(output_dir, device_ids):
        import jax
        jax.devices()
        if device_ids:
            ids = (ctypes.c_int64 * len(device_ids))(*device_ids)
            rc = lib.axon_start_nrt_profile(ids, len(device_ids))
        else:
            rc = lib.axon_start_nrt_profile(None, 0)
        if rc != 0:
            raise RuntimeError(f"axon_start_nrt_profile rc={rc}")
        try:
            yield
        finally:
            n = lib.axon_stop_nrt_profile(str(output_dir).encode())
            print(f"profile: {n} file(s) written to {output_dir}")

    mod = types.ModuleType("antenv.axon_hooks")
    mod.get_axon_ntff_profile_hook = lambda: _hook
    mod.set_axon_ntff_profile_hook = lambda h: None
    sys.modules["antenv.axon_hooks"] = mod


def _split16(c):
    """Split float64 array c into (hi, lo) fp16 with lo pre-scaled by 2^11."""
    hi = c.astype(np.float16)
    lo = ((c - hi.astype(np.float64)) * LO_SCALE).astype(np.float16)
    return hi, lo


def kernel(xyz, cholesky, opacity, features_dc):
    from concourse import bass_utils

    xyz = np.asarray(xyz, np.float32)
    cholesky = np.asarray(cholesky, np.float32)
    opacity = np.asarray(opacity, np.float32)
    features_dc = np.asarray(features_dc, np.float32)

    # ---- host precompute (float64): projection, conic, binning ----
    means = np.tanh(xyz.astype(np.float64))
    cx = 0.5 * W * (means[..., 0] + 1.0)                    # (T,N)
    cy = 0.5 * H * (means[..., 1] + 1.0)
    chol = cholesky.astype(np.float64) + np.array([0.5, 0.0, 0.5])
    l0, l1, l2 = chol[..., 0], chol[..., 1], chol[..., 2]
    sxx, sxy, syy = l0 * l0, l0 * l1, l1 * l1 + l2 * l2
    det = sxx * syy - sxy * sxy
    ca, cb, cc = syy / det, -sxy / det, sxx / det           # conic (T,N)
    tr = sxx + syy
    lam = tr / 2 + np.sqrt(np.maximum(tr * tr / 4 - det, 0.0))

    colors = 1.0 / (1.0 + np.exp(-features_dc.astype(np.float64)))   # (N,3)
    opac = 1.0 / (1.0 + np.exp(-opacity.astype(np.float64)[:, 0]))   # (N,)
    w3 = colors * opac[:, None]                                      # (N,3)

    entries = _bin_entries(cx, cy, lam)
    total = sum(len(e[3]) for e in entries)
    E = max(2, -(-total // (SLOTS * N_CORES)))   # bins per core, lower bound
    bins = None
    while bins is None:
        bins = _pack_bins(entries, E)
        if bins is None:
            E += 1

    # fp16 quadratic basis over local 32x32 pixels; rows 6-11 are the
    # lo-coefficient rows, scaled by 2^-11 (power of two: still exact)
    gx = np.arange(PIX, dtype=np.float64) % TILE
    gy = np.arange(PIX, dtype=np.float64) // TILE
    b6 = np.stack([gx * gx, gx * gy, gy * gy, gx, gy, np.ones(PIX)])
    basis = np.concatenate([b6, b6 / LO_SCALE]).astype(np.float16)

    in_maps = []
    for c in range(N_CORES):
        lm = np.zeros((12, E * SLOTS), np.float16)
        wm = np.zeros((SLOTS, E * 32), np.float16)
        for ei in range(E):
            off = 0
            for j, k in enumerate(bins[c * E + ei]):
                t, ty, tx, idxs = entries[k]
                ns = len(idxs)
                if not ns:
                    continue
                idxs = np.asarray(idxs)
                ex = cx[t, idxs] - tx * TILE
                ey = cy[t, idxs] - ty * TILE
                a_, b_, c_ = ca[t, idxs], cb[t, idxs], cc[t, idxs]
                coef = np.stack([
                    0.5 * a_,
                    b_,
                    0.5 * c_,
                    -(a_ * ex + b_ * ey),
                    -(b_ * ex + c_ * ey),
                    0.5 * (a_ * ex * ex + c_ * ey * ey) + b_ * ex * ey,
                ])                                           # (6, ns)
                hi, lo = _split16(coef)
                s = slice(SLOTS * ei + off, SLOTS * ei + off + ns)
                lm[0:6, s] = hi
                lm[6:12, s] = lo
                wm[off:off + ns, 32 * ei + 3 * j:32 * ei + 3 * j + 3] = \
                    w3[idxs].astype(np.float16)
                off += ns
        cbam = np.concatenate([basis[:, 0:512], lm[:, 0:SLOTS]],
                              axis=1).astype(np.float16)
        cbsm = np.ascontiguousarray(basis[:, 512:1024])
        cbbm = np.ascontiguousarray(lm[:, SLOTS:]) if E > 1 else \
            np.zeros((12, SLOTS), np.float16)
        in_maps.append({"cba": cbam, "cbs": cbsm, "cbb": cbbm, "wtr": wm})

    if E not in _CACHE:
        _CACHE[E] = _build_nc(E)
    nc = _CACHE[E]

    trace = bool(int(os.environ.get("GS_TRACE", "0")))
    if trace:
        _ensure_ntff_hook()
    res = bass_utils.run_bass_kernel_spmd(
        nc, in_maps, core_ids=list(range(N_CORES)), trace=trace)
    kernel.last_result = res

    img = np.zeros((T, 3, H, W), np.float32)
    for c in range(N_CORES):
        o = res.results[c]["out"]     # (G, 128, PIX) fp16
        for ei in range(E):
            g, i = divmod(ei, 4)
            for j, k in enumerate(bins[c * E + ei]):
                t, ty, tx, _ = entries[k]
                blk = o[g, 32 * i + 3 * j:32 * i + 3 * j + 3]
                img[t, :, ty * TILE:(ty + 1) * TILE,
                    tx * TILE:(tx + 1) * TILE] = \
                    blk.reshape(3, TILE, TILE)
    return np.clip(img, 0.0, 1.0)


# revision 28
# speedup vs baseline: 4.8195x; 1.0100x over previous
"""GaussianImage (Cholesky) renderer on 8 trn2 NeuronCores.

Strategy: tile-parallel over the pixel grid with multi-tile slot packing.
The 256x256 image is cut into 32x32-pixel tiles (64/frame, 128 total for
T=2).  The host bins gaussians to tiles (bbox intersect via a
conservative support radius; outside it exp(-sigma) underflows to 0 in
fp32) and then bin-packs several tiles into one 128-slot "group"
(occupancies sum to <= 128; mean tile occupancy is ~40).  All tiles share
the same local 32x32 quadratic basis, so one K=12 fp16 matmul + one Exp
evaluates every gaussian of every tile in the group against all 1024
local pixels:

  sigma = lhsT(12,128)^T @ basis(12,1024)      [TensorE fp16, fp32 PSUM]
  alpha = Exp(-sigma)                          [ScalarE -> fp16]
  img   = w(128,32)^T @ alpha(128,1024)        [TensorE fp16, fp32 PSUM]

The img weights are block-structured: column 3j+c holds channel-c colors
of tile j's gaussians at their slots and zeros elsewhere, so one matmul
renders every tile of the group (row 3j+c = tile j, channel c).  Per-slot
sigma coefficients (quadratic in local pixel coords) are precomputed on
the host and split hi/lo into two fp16 values (lo pre-scaled by 2^11 to
stay in fp16 normal range; matching basis rows scaled by 2^-11), giving
~fp32-accurate sigma at the PE's full fp16 rate.  Images for 4 groups
land in one PSUM tile at partition offsets 0/32/64/96 (PE column-group
tiling), are clamped+converted to fp16 by one VectorE op, and DMA'd out.
Each pixel is owned by exactly one tile -> no cross-core reduction.
"""

import os
import numpy as np

T, N, H, W = 2, 512, 256, 256
TILE = 32
NT = H // TILE          # 8 tiles per axis
N_CORES = 8
SLOTS = 128
PIX = TILE * TILE       # 1024
MAXTILES = 10           # 3*MAXTILES <= 32 img-weight columns per group
SIGMA_CUT = 20.0        # exp(-20) ~ 2e-9: negligible vs the 2e-2 rel gate
LO_SCALE = 2048.0       # 2^11: keeps lo-half fp16 coefficients normal

_CACHE = {}


def _build_nc(E):
    import concourse.bass as bass
    import concourse.mybir as mybir
    from concourse.tile import TileContext
    import bass_rust

    f32 = mybir.dt.float32
    f16 = mybir.dt.float16
    Act = mybir.ActivationFunctionType
    Alu = mybir.AluOpType
    G = (E + 3) // 4  # img PSUM tiles, 4 groups each

    nc = bass.Bass("TRN2")
    # Inputs split into four tensors so each rides a different engine's
    # DMA queue and the transfers run in parallel right after engine init:
    #   cba: [basis cols 0:512 | group-0 lhsT coeffs]   (sync queue)
    #   cbs: [basis cols 512:1024]                      (tensor queue)
    #   cbb: remaining groups' lhsT coeffs              (vector queue)
    #   wtr: img weights                                (scalar queue)
    cba = nc.dram_tensor("cba", [12, 512 + SLOTS], f16, kind="ExternalInput")
    cbs = nc.dram_tensor("cbs", [12, 512], f16, kind="ExternalInput")
    cbb = nc.dram_tensor("cbb", [12, max(E - 1, 1) * SLOTS], f16,
                         kind="ExternalInput")
    wtr = nc.dram_tensor("wtr", [SLOTS, E * 32], f16, kind="ExternalInput")
    out = nc.dram_tensor("out", [G, SLOTS, PIX], f16, kind="ExternalOutput")

    with TileContext(nc) as tc:
        with tc.tile_pool(name="const", bufs=1) as cpool, \
             tc.tile_pool(name="alpha", bufs=4) as apool, \
             tc.tile_pool(name="ps_sig", bufs=3 if G == 1 else 2,
                          space="PSUM") as pss, \
             tc.tile_pool(name="ps_img", bufs=2 if G > 1 else 1,
                          space="PSUM") as psi:

            S = nc.scalar

            # preload the Exp activation table first thing on the scalar
            # queue (the ~1.3us table load otherwise serializes with the
            # first real exp).  The warm tile is read uninitialized on
            # purpose: its output is never consumed.
            warm = cpool.tile([SLOTS, 1], f32, tag="warm")
            S.activation(warm, warm, Act.Exp)

            cta = cpool.tile([12, 512 + SLOTS], f16, tag="cba")
            cts = cpool.tile([12, 512], f16, tag="cbs")
            ctb = cpool.tile([12, max(E - 1, 1) * SLOTS], f16, tag="cbb")
            wt = cpool.tile([SLOTS, E * 32], f16, tag="wtr")
            nc.scalar.dma_start(out=wt, in_=wtr[:])
            nc.sync.dma_start(out=cta, in_=cba[:])
            nc.gpsimd.dma_start(out=cts, in_=cbs[:])
            nc.sync.dma_start(out=ctb, in_=cbb[:])

            def bt(c0, c1):
                return cta[:, c0:c1] if c1 <= 512 else cts[:, c0 - 512:c1 - 512]

            alphas = {}
            imgt = {}

            def emit_img(e):
                g, i = divmod(e, 4)
                if i == 0:
                    # separate single-bank PSUM tiles per 512-col half so
                    # the drain of one half never serializes against img
                    # matmuls of the other (the bank tracker is per-tile)
                    imgt[g] = (psi.tile([SLOTS, 512], f32, tag="imgA",
                                        name=f"imgA{g}"),
                               psi.tile([SLOTS, 512], f32, tag="imgB",
                                        name=f"imgB{g}"))
                al = alphas.pop(e)
                wre = wt[:, 32 * e:32 * e + 32]
                final = e == E - 1
                last = final or i == 3
                for h, (c0, c1) in enumerate(((0, 512), (512, 1024))):
                    t = imgt[g][h]
                    nc.tensor.matmul(t[32 * i:32 * i + 32, :], wre,
                                     al[:, c0:c1], start=True, stop=True,
                                     tile_position=(0, 32 * i))
                    if last:
                        # fused clamp + PSUM->SBUF fp16 copy per half (one
                        # DVE op covers all groups in the block: cost is
                        # cols, not rows), then DMA out
                        st = apool.tile([SLOTS, 512], f16, tag=f"st{h}",
                                        name=f"st{g}_{h}")
                        nc.vector.tensor_scalar(out=st, in0=t,
                                                scalar1=0.0, scalar2=1.0,
                                                op0=Alu.max, op1=Alu.min)
                        nc.sync.dma_start(out=out[g][:, c0:c1], in_=st)

            for e in range(E):
                sig = pss.tile([SLOTS, PIX], f32, tag="sig", name=f"sig{e}")
                lh = cta[:, 512:512 + SLOTS] if e == 0 else \
                    ctb[:, SLOTS * (e - 1):SLOTS * e]
                nc.tensor.matmul(sig[:, 0:512], lh, bt(0, 512),
                                 start=True, stop=True)
                nc.tensor.matmul(sig[:, 512:1024], lh, bt(512, 1024),
                                 start=True, stop=True)
                # software pipeline with a lag of 2 groups: by the time the
                # img matmuls of group e-2 issue, its exp finished long ago,
                # so the PE never stalls on the ScalarE
                if e >= 2:
                    emit_img(e - 2)
                al = apool.tile([SLOTS, PIX], f16, tag="alpha", name=f"al{e}")
                if e == 0 or e == E - 1:
                    # split the first exp (starts right after the first sig
                    # half-matmul, pulling the whole ScalarE chain earlier)
                    # and the final exp (pipelines the drain chain
                    # sig->exp->img->clamp->dma at 512-col grain)
                    S.activation(al[:, 0:512], sig[:, 0:512], Act.Exp,
                                 scale=-1.0)
                    S.activation(al[:, 512:1024], sig[:, 512:1024], Act.Exp,
                                 scale=-1.0)
                else:
                    S.activation(al, sig, Act.Exp, scale=-1.0)
                alphas[e] = al
            if E >= 2:
                emit_img(E - 2)
            emit_img(E - 1)

    bass_rust.generate_event_semaphores(nc)
    return nc


def _bin_entries(cx, cy, lam):
    """Host-side routing: which gaussians overlap which 32x32 tile."""
    r = np.sqrt(2.0 * SIGMA_CUT * np.maximum(lam, 0.0)) + 1.0

    entries = []  # (frame, ty, tx, index-list)
    for t in range(T):
        x0 = np.clip(((cx[t] - r[t]) // TILE).astype(int), 0, NT - 1)
        x1 = np.clip(((cx[t] + r[t]) // TILE).astype(int), 0, NT - 1)
        y0 = np.clip(((cy[t] - r[t]) // TILE).astype(int), 0, NT - 1)
        y1 = np.clip(((cy[t] + r[t]) // TILE).astype(int), 0, NT - 1)
        buckets = [[[] for _ in range(NT)] for _ in range(NT)]
        for n in range(N):
            for ty in range(y0[n], y1[n] + 1):
                for tx in range(x0[n], x1[n] + 1):
                    buckets[ty][tx].append(n)
        for ty in range(NT):
            for tx in range(NT):
                assert len(buckets[ty][tx]) <= SLOTS, "tile overflow: >128 gaussians"
                entries.append((t, ty, tx, buckets[ty][tx]))
    return entries


def _pack_bins(entries, bins_per_core):
    """Pack tile-entries into groups of <= 128 total slots, <= MAXTILES
    tiles, load-balanced over N_CORES * bins_per_core bins. Returns a list
    of bins, each a list of entry indices, or None if infeasible."""
    nbins = N_CORES * bins_per_core
    order = sorted(range(len(entries)),
                   key=lambda k: -len(entries[k][3]))
    loads = [0] * nbins
    counts = [0] * nbins
    bins = [[] for _ in range(nbins)]
    for k in order:
        occ = len(entries[k][3])
        best = -1
        for b in sorted(range(nbins), key=lambda b: loads[b]):
            if counts[b] < MAXTILES and loads[b] + occ <= SLOTS:
                best = b
                break
        if best < 0:
            return None
        bins[best].append(k)
        loads[best] += occ
        counts[best] += 1
    return bins


def _ensure_ntff_hook():
    """Provide antenv.axon_hooks (missing in this image) so trace=True works."""
    import sys, types, ctypes, contextlib
    if "antenv.axon_hooks" in sys.modules:
        return
    so_path = "/opt/axon/libaxon_pjrt.so"
    if not os.path.exists(so_path):
        return
    lib = ctypes.CDLL(so_path)
    if not hasattr(lib, "axon_start_nrt_profile"):
        return
    lib.axon_start_nrt_profile.argtypes = [ctypes.POINTER(ctypes.c_int64), ctypes.c_size_t]
    lib.axon_start_nrt_profile.restype = ctypes.c_int64
    lib.axon_stop_nrt_profile.argtypes = [ctypes.c_char_p]
    lib.axon_stop_nrt_profile.restype = ctypes.c_int64

    @contextlib.contextmanager
    def _hook(output_dir, device_ids):
        import jax
        jax.devices()
        if device_ids:
            ids = (ctypes.c_int64 * len(device_ids))(*device_ids)
            rc = lib.axon_start_nrt_profile(ids, len(device_ids))
        else:
            rc = lib.axon_start_nrt_profile(None, 0)
        if rc != 0:
            raise RuntimeError(f"axon_start_nrt_profile rc={rc}")
        try:
            yield
        finally:
            n = lib.axon_stop_nrt_profile(str(output_dir).encode())
            print(f"profile: {n} file(s) written to {output_dir}")

    mod = types.ModuleType("antenv.axon_hooks")
    mod.get_axon_ntff_profile_hook = lambda: _hook
    mod.set_axon_ntff_profile_hook = lambda h: None
    sys.modules["antenv.axon_hooks"] = mod


def _split16(c):
    """Split float64 array c into (hi, lo) fp16 with lo pre-scaled by 2^11."""
    hi = c.astype(np.float16)
    lo = ((c - hi.astype(np.float64)) * LO_SCALE).astype(np.float16)
    return hi, lo


def kernel(xyz, cholesky, opacity, features_dc):
    from concourse import bass_utils

    xyz = np.asarray(xyz, np.float32)
    cholesky = np.asarray(cholesky, np.float32)
    opacity = np.asarray(opacity, np.float32)
    features_dc = np.asarray(features_dc, np.float32)

    # ---- host precompute (float64): projection, conic, binning ----
    means = np.tanh(xyz.astype(np.float64))
    cx = 0.5 * W * (means[..., 0] + 1.0)                    # (T,N)
    cy = 0.5 * H * (means[..., 1] + 1.0)
    chol = cholesky.astype(np.float64) + np.array([0.5, 0.0, 0.5])
    l0, l1, l2 = chol[..., 0], chol[..., 1], chol[..., 2]
    sxx, sxy, syy = l0 * l0, l0 * l1, l1 * l1 + l2 * l2
    det = sxx * syy - sxy * sxy
    ca, cb, cc = syy / det, -sxy / det, sxx / det           # conic (T,N)
    tr = sxx + syy
    lam = tr / 2 + np.sqrt(np.maximum(tr * tr / 4 - det, 0.0))

    colors = 1.0 / (1.0 + np.exp(-features_dc.astype(np.float64)))   # (N,3)
    opac = 1.0 / (1.0 + np.exp(-opacity.astype(np.float64)[:, 0]))   # (N,)
    w3 = colors * opac[:, None]                                      # (N,3)

    entries = _bin_entries(cx, cy, lam)
    total = sum(len(e[3]) for e in entries)
    E = max(2, -(-total // (SLOTS * N_CORES)))   # bins per core, lower bound
    bins = None
    while bins is None:
        bins = _pack_bins(entries, E)
        if bins is None:
            E += 1

    # fp16 quadratic basis over local 32x32 pixels; rows 6-11 are the
    # lo-coefficient rows, scaled by 2^-11 (power of two: still exact)
    gx = np.arange(PIX, dtype=np.float64) % TILE
    gy = np.arange(PIX, dtype=np.float64) // TILE
    b6 = np.stack([gx * gx, gx * gy, gy * gy, gx, gy, np.ones(PIX)])
    basis = np.concatenate([b6, b6 / LO_SCALE]).astype(np.float16)

    in_maps = []
    for c in range(N_CORES):
        lm = np.zeros((12, E * SLOTS), np.float16)
        wm = np.zeros((SLOTS, E * 32), np.float16)
        for ei in range(E):
            off = 0
            for j, k in enumerate(bins[c * E + ei]):
                t, ty, tx, idxs = entries[k]
                ns = len(idxs)
                if not ns:
                    continue
                idxs = np.asarray(idxs)
                ex = cx[t, idxs] - tx * TILE
                ey = cy[t, idxs] - ty * TILE
                a_, b_, c_ = ca[t, idxs], cb[t, idxs], cc[t, idxs]
                coef = np.stack([
                    0.5 * a_,
                    b_,
                    0.5 * c_,
                    -(a_ * ex + b_ * ey),
                    -(b_ * ex + c_ * ey),
                    0.5 * (a_ * ex * ex + c_ * ey * ey) + b_ * ex * ey,
                ])                                           # (6, ns)
                hi, lo = _split16(coef)
                s = slice(SLOTS * ei + off, SLOTS * ei + off + ns)
                lm[0:6, s] = hi
                lm[6:12, s] = lo
                wm[off:off + ns, 32 * ei + 3 * j:32 * ei + 3 * j + 3] = \
                    w3[idxs].astype(np.float16)
                off += ns
        cbam = np.concatenate([basis[:, 0:512], lm[:, 0:SLOTS]],
                              axis=1).astype(np.float16)
        cbsm = np.ascontiguousarray(basis[:, 512:1024])
        cbbm = np.ascontiguousarray(lm[:, SLOTS:]) if E > 1 else \
            np.zeros((12, SLOTS), np.float16)
        in_maps.append({"cba": cbam, "cbs": cbsm, "cbb": cbbm, "wtr": wm})

    if E not in _CACHE:
        _CACHE[E] = _build_nc(E)
    nc = _CACHE[E]

    trace = bool(int(os.environ.get("GS_TRACE", "0")))
    if trace:
        _ensure_ntff_hook()
    res = bass_utils.run_bass_kernel_spmd(
        nc, in_maps, core_ids=list(range(N_CORES)), trace=trace)
    kernel.last_result = res

    img = np.zeros((T, 3, H, W), np.float32)
    for c in range(N_CORES):
        o = res.results[c]["out"]     # (G, 128, PIX) fp16
        for ei in range(E):
            g, i = divmod(ei, 4)
            for j, k in enumerate(bins[c * E + ei]):
                t, ty, tx, _ = entries[k]
                blk = o[g, 32 * i + 3 * j:32 * i + 3 * j + 3]
                img[t, :, ty * TILE:(ty + 1) * TILE,
                    tx * TILE:(tx + 1) * TILE] = \
                    blk.reshape(3, TILE, TILE)
    return np.clip(img, 0.0, 1.0)
